# revision 3
# baseline (speedup 1.0000x reference)
"""Trainium2 Bass kernel for nn_BlockSparseMoE (top-2 of 8 experts, SwiGLU).

Strategy (8-way tensor-parallel over FFN):
  - Host: compute router (gate matmul + softmax + top-2 + renorm) in fp64,
    gather each expert's tokens into a contiguous column range of one
    shared xT matrix.
  - Device (SPMD x8): every core holds a 512-wide F-slice of ALL 8
    experts' w1/w3/w2 (same ~25 MB weight traffic as one full expert in
    the expert-parallel layout) and runs all 8192 token-expert pairs
    against its slice — exactly T*K/8 = 1024 pair-equivalents per core
    regardless of routing imbalance. Partial y outputs (transposed,
    unscaled) stream back.
  - Host: sum the 8 partial outputs, scale by the renormalized top-2
    weight, scatter-add per token.

Per-core layout:
  phase A: hT[f, t] = silu(x@w1)^T * (x@w3)^T per expert (FT=4 f-tiles of
           128), lhsT = w1 tile [128, 128f], rhs = xT d-chunk
           [128, tchunk] — weights stationary, tokens moving.
  phase B: yT[d, t] = w2_slice^T @ hT, lhsT = w2 f-tile [128f, 128d],
           rhs = hT f-tile [128, tchunk] — tokens moving, so ragged
           expert tails cost no extra PE cycles; no on-device scaling.

Startup: expert 0's weights are pre-tiled per f-tile ([128, DC, 128]
  each, 256 KB) and its first chunk is only 256 cols, so the first real
  matmul's inputs (~1 MB of DMA) land ~5us earlier than a monolithic
  load; a short HAM warmup bridges kernel entry to first-weights.
"""

import numpy as np
import ml_dtypes

HIDDEN = 1024
FFN = 4096
NUM_EXPERTS = 8
TOP_K = 2
N_CORES = 8
FS = FFN // N_CORES          # 512-wide F-slice per core
DC = HIDDEN // 128           # 8 contraction chunks for x@w1
FT = FS // 128               # 4 f-tiles per expert slice
DT = HIDDEN // 128           # 8 output d-tiles

_BF16 = ml_dtypes.bfloat16
_nc_cache = {}


# ---------------------------------------------------------------- router ----
def _route(x, gate_w, gate_b):
    """Top-2 routing. Returns per-expert (token_idx, renorm_weight)."""
    logits = x.astype(np.float64) @ gate_w.astype(np.float64) + gate_b.astype(
        np.float64
    )
    logits -= logits.max(axis=-1, keepdims=True)
    p = np.exp(logits)
    p /= p.sum(axis=-1, keepdims=True)
    # top-2 by prob, ties broken by lower index (matches jax.lax.top_k)
    top2 = np.argsort(-p, axis=-1, kind="stable")[:, :TOP_K]
    pt = np.take_along_axis(p, top2, axis=-1)
    wt = pt / pt.sum(axis=-1, keepdims=True)
    idxs, wts = [], []
    for e in range(NUM_EXPERTS):
        mask = top2 == e  # [T, 2]
        tok = np.nonzero(mask.any(axis=-1))[0]
        w = wt[tok, np.argmax(mask[tok], axis=-1)]
        idxs.append(tok)
        wts.append(w.astype(np.float32))
    return idxs, wts


def _chunks_for(load):
    """Split a token count into moving-dim chunks: all but the last are
    multiples of 128 in [256, 512]; keep the ragged tail >= 240 when
    possible (short moving dims go LDWEIGHTS-bound)."""
    C = load
    n = max(1, -(-C // 512))
    chunks = []
    rem = C
    for i in range(n - 1):
        c = min(512, -(-rem // ((n - i) * 128)) * 128)
        chunks.append(c)
        rem -= c
    while n > 1 and rem < 240 and chunks:
        for i in range(len(chunks)):
            if rem >= 240:
                break
            if chunks[i] > 256:
                chunks[i] -= 128
                rem += 128
        else:
            break
    chunks.append(rem)
    assert sum(chunks) == C and all(c > 0 for c in chunks)
    return tuple(chunks)


def _chunks_first(load):
    """Expert 0 gets a small 256-col first chunk so the first matmul's
    xT DMA is small; 256 is the LDWEIGHTS/matmul breakeven width."""
    if load >= 640:
        return (256,) + _chunks_for(load - 256)
    return _chunks_for(load)


def _plan(loads):
    """Per-expert chunk tuples + 128-aligned xT column offsets."""
    chunks_e, offs = [], []
    off = 0
    for e, l in enumerate(loads):
        chunks_e.append(_chunks_first(l) if e == 0 else _chunks_for(l))
        offs.append(off)
        off += -(-l // 128) * 128
    return tuple(chunks_e), tuple(offs), off


# ------------------------------------------------------------- device IR ----
def _build(plan):
    """Per-core Bacc graph. plan = (chunks_e, offs, XWT)."""
    import concourse.bacc as bacc
    import concourse.bass as bass
    import concourse.mybir as mybir
    import concourse.tile as tile

    chunks_e, offs, XWT = plan
    XW_e = [-(-sum(ch) // 128) * 128 for ch in chunks_e]

    bf16 = mybir.dt.bfloat16
    f32 = mybir.dt.float32

    nc = bacc.Bacc("TRN2", target_bir_lowering=False, debug=False,
                   num_devices=N_CORES)

    xT_d = nc.dram_tensor("xT", [HIDDEN, XWT], bf16, kind="ExternalInput")
    # w1s/w3s host-pre-tiled per expert as [e, p, ft, dc, 128]; w2s as
    # [e, p, ft, HIDDEN] so every DMA line is fully contiguous and any
    # single f-tile of an expert is one contiguous 2KB/partition run
    w1_d = nc.dram_tensor("w1s", [NUM_EXPERTS, 128, FT, DC, 128], bf16,
                          kind="ExternalInput")
    w3_d = nc.dram_tensor("w3s", [NUM_EXPERTS, 128, FT, DC, 128], bf16,
                          kind="ExternalInput")
    w2_d = nc.dram_tensor("w2s", [NUM_EXPERTS, 128, FT, HIDDEN], bf16,
                          kind="ExternalInput")
    y_d = nc.dram_tensor("y", [HIDDEN, XWT], bf16, kind="ExternalOutput")

    xT_v = xT_d.ap().rearrange("(dc p) c -> p dc c", p=128)
    y_v = y_d.ap().rearrange("(dt p) c -> dt p c", p=128)

    with tile.TileContext(nc) as tc:
        with (
            tc.tile_pool(name="xe", bufs=2) as xep,
            tc.tile_pool(name="w13", bufs=2) as w13,
            tc.tile_pool(name="w2p", bufs=2) as w2p,
            tc.tile_pool(name="hp", bufs=2) as hp,
            tc.tile_pool(name="sil", bufs=4) as silp,
            tc.tile_pool(name="yo", bufs=8) as yop,
            tc.tile_pool(name="yq", bufs=4) as yqp,
            tc.tile_pool(name="ps", bufs=2, space=bass.MemorySpace.PSUM) as ps,
            tc.tile_pool(name="yps", bufs=4, space=bass.MemorySpace.PSUM) as yps,
        ):
            xe_tiles = {}
            w13_tiles = {}
            w2_tiles = {}

            def load_w13(e):
                w1_sb = w13.tile([128, FT, DC, 128], bf16, tag="w1",
                                 name="w1_sb")
                w3_sb = w13.tile([128, FT, DC, 128], bf16, tag="w3",
                                 name="w3_sb")
                nc.sync.dma_start(w1_sb[:], w1_d.ap()[e])
                nc.sync.dma_start(w3_sb[:], w3_d.ap()[e])
                w13_tiles[e] = (w1_sb, w3_sb)

            def load_xe(e, col0=0, col1=None):
                if e not in xe_tiles:
                    xe_tiles[e] = xep.tile([128, DC, XW_e[e]], bf16,
                                           tag="xT", name="xe_sb")
                if col1 is None:
                    col1 = XW_e[e]
                nc.sync.dma_start(
                    xe_tiles[e][:, :, col0:col1],
                    xT_v[:, :, offs[e] + col0:offs[e] + col1],
                )

            def load_w2(e):
                w2_sb = w2p.tile([128, FT, HIDDEN], bf16, tag="w2",
                                 name="w2_sb")
                nc.sync.dma_start(w2_sb[:], w2_d.ap()[e])
                w2_tiles[e] = w2_sb

            # ---- startup: expert 0's weights land one f-tile at a time so
            # the first matmul chain (ft0: 8 w1-dc tiles + xe chunk0) only
            # waits on ~1MB of DMA instead of 3MB.
            c0 = chunks_e[0][0]
            w1f = [w13.tile([128, DC, 128], bf16, tag=f"w1f{ft}", bufs=1,
                            name=f"w1f{ft}") for ft in range(FT)]
            w3f = [w13.tile([128, DC, 128], bf16, tag=f"w3f{ft}", bufs=1,
                            name=f"w3f{ft}") for ft in range(FT)]
            xe_tiles[0] = xep.tile([128, DC, XW_e[0]], bf16, tag="xT",
                                   name="xe_sb0")
            nc.sync.dma_start(xe_tiles[0][:, :, 0:c0], xT_v[:, :, 0:c0])
            for ft in range(FT):
                nc.sync.dma_start(w1f[ft][:], w1_d.ap()[0][:, ft])
                nc.sync.dma_start(w3f[ft][:], w3_d.ap()[0][:, ft])
            load_w2(0)
            if XW_e[0] > c0:
                load_xe(0, col0=c0, col1=min(XW_e[0], c0 + 512))
            if XW_e[0] > c0 + 512:
                load_xe(0, col0=c0 + 512)

            def w1slice(e, ft, dc):
                if e == 0:
                    return w1f[ft][:, dc, :]
                return w13_tiles[e][0][:, ft, dc, :]

            def w3slice(e, ft, dc):
                if e == 0:
                    return w3f[ft][:, dc, :]
                return w13_tiles[e][1][:, ft, dc, :]

            # HAM pre-warm: keep the PE's activity monitor busy from kernel
            # entry (~6.4us) until the first real matmul's inputs land
            # (~8.7us) so the clock is ramping before real work starts.
            warm_sb = silp.tile([128, 128], bf16, tag="warm_in", bufs=1)
            nc.gpsimd.memset(warm_sb[:], 0.0)
            warm_ps = ps.tile([128, 128], f32, tag="ph1", name="warm_ps")
            N_WARM = 24
            for i in range(N_WARM):
                nc.tensor.matmul(warm_ps[:], warm_sb[:], warm_sb[:],
                                 start=(i == 0), stop=(i == N_WARM - 1))

            for e in range(NUM_EXPERTS):
                t0 = 0
                nch = len(chunks_e[e])
                # which chunk carries the next-expert prefetches: for e0
                # use the second chunk (chunk0 is tiny and its DMA window
                # is already packed with e0's own loads)
                pf_ci = (1 if nch > 1 else 0) if e == 0 else 0
                for ci, chunk in enumerate(chunks_e[e]):
                    xe = xe_tiles[e]
                    hT = hp.tile([128, FT, chunk], bf16, tag="hT")
                    # ---- phase A ----
                    for ft in range(FT):
                        # prefetches ride behind this chunk's compute
                        if ci == pf_ci and e + 1 < NUM_EXPERTS:
                            if ft == 2:
                                load_w13(e + 1)
                            elif ft == 3:
                                load_xe(e + 1)
                                load_w2(e + 1)
                        ph1 = ps.tile([128, chunk], f32, tag="ph1")
                        ph3 = ps.tile([128, chunk], f32, tag="ph3")
                        for dc in range(DC):
                            nc.tensor.matmul(
                                ph1[:],
                                w1slice(e, ft, dc),
                                xe[:, dc, t0:t0 + chunk],
                                start=(dc == 0), stop=(dc == DC - 1),
                            )
                        for dc in range(DC):
                            nc.tensor.matmul(
                                ph3[:],
                                w3slice(e, ft, dc),
                                xe[:, dc, t0:t0 + chunk],
                                start=(dc == 0), stop=(dc == DC - 1),
                            )
                        sil = silp.tile([128, chunk], bf16, tag="sil")
                        nc.scalar.activation(
                            sil[:], ph1[:], mybir.ActivationFunctionType.Silu
                        )
                        nc.vector.tensor_mul(hT[:, ft, :], sil[:], ph3[:])

                    # ---- phase B: yT[d, t] partial, unscaled ----
                    w2_sb = w2_tiles[e]
                    is_last = (e == NUM_EXPERTS - 1
                               and ci == len(chunks_e[e]) - 1)
                    for dt in range(DT):
                        yp = yps.tile([128, chunk], f32, tag="yp")
                        for ft in range(FT):
                            nc.tensor.matmul(
                                yp[:],
                                w2_sb[:, ft, dt * 128:(dt + 1) * 128],
                                hT[:, ft, :],
                                start=(ft == 0), stop=(ft == FT - 1),
                            )
                        # alternate copies between ScalarE and DVE so
                        # neither engine becomes the bottleneck; on the
                        # very last output tile, drain in column-quarters
                        # on BOTH engines concurrently (separate tiles →
                        # no WAW serialization) so only a tiny copy+DMA
                        # trails the final matmul
                        col0 = offs[e] + t0
                        if is_last and dt == DT - 1:
                            q = -(-chunk // 4)
                            for p0 in range(0, chunk, q):
                                p1 = min(chunk, p0 + q)
                                ysb = yqp.tile([128, q], bf16, tag="yq")
                                if (p0 // q) % 2 == 0:
                                    nc.scalar.copy(ysb[:, 0:p1 - p0],
                                                   yp[:, p0:p1])
                                else:
                                    nc.vector.tensor_copy(ysb[:, 0:p1 - p0],
                                                          yp[:, p0:p1])
                                nc.sync.dma_start(
                                    y_v[dt][:, col0 + p0:col0 + p1],
                                    ysb[:, 0:p1 - p0],
                                )
                        else:
                            ysb = yop.tile([128, chunk], bf16, tag="ysb")
                            if dt % 2 == 0:
                                nc.scalar.copy(ysb[:], yp[:])
                            else:
                                nc.vector.tensor_copy(ysb[:], yp[:])
                            nc.sync.dma_start(
                                y_v[dt][:, col0:col0 + chunk],
                                ysb[:],
                            )
                    t0 += chunk
    nc.compile()
    return nc


def _get_nc(plan):
    if plan not in _nc_cache:
        _nc_cache[plan] = _build(plan)
    return _nc_cache[plan]


# ---------------------------------------------------------------- kernel ----
def kernel(hidden_states, gate_w, gate_b, w1, w3, w2, _trace=False):
    from concourse.bass_utils import run_bass_kernel_spmd

    B, S, D = hidden_states.shape
    T = B * S
    x = np.asarray(hidden_states, np.float32).reshape(T, D)
    idxs, wts = _route(x, np.asarray(gate_w, np.float32),
                       np.asarray(gate_b, np.float32))
    loads = [len(i) for i in idxs]
    plan = _plan(loads)
    chunks_e, offs, XWT = plan
    nc = _get_nc(plan)

    # shared xT: every expert's tokens in its 128-aligned column range
    xT = np.zeros((D, XWT), _BF16)
    for e in range(NUM_EXPERTS):
        xT[:, offs[e]:offs[e] + loads[e]] = x[idxs[e]].T.astype(_BF16)

    # weights pre-tiled for all cores in one reshape/transpose:
    # w1/w3 [E, D, F] -> [core, E, 128, ft, dc, 128]
    w1 = np.asarray(w1, np.float32)
    w3 = np.asarray(w3, np.float32)
    w2 = np.asarray(w2, np.float32)
    w1t = np.ascontiguousarray(
        w1.reshape(NUM_EXPERTS, DC, 128, N_CORES, FT, 128)
        .transpose(3, 0, 2, 4, 1, 5)).astype(_BF16)
    w3t = np.ascontiguousarray(
        w3.reshape(NUM_EXPERTS, DC, 128, N_CORES, FT, 128)
        .transpose(3, 0, 2, 4, 1, 5)).astype(_BF16)
    # w2 [E, F, D] -> [core, E, 128, ft, D]
    w2t = np.ascontiguousarray(
        w2.reshape(NUM_EXPERTS, N_CORES, FT, 128, HIDDEN)
        .transpose(1, 0, 3, 2, 4)).astype(_BF16)

    in_maps = [{
        "xT": xT,
        "w1s": w1t[c],
        "w3s": w3t[c],
        "w2s": w2t[c],
    } for c in range(N_CORES)]

    res = run_bass_kernel_spmd(nc, in_maps, core_ids=list(range(N_CORES)),
                               trace=_trace)

    yT = res.results[0]["y"].astype(np.float32)
    for c in range(1, N_CORES):
        yT += res.results[c]["y"].astype(np.float32)
    out = np.zeros((T, D), np.float32)
    for e in range(NUM_EXPERTS):
        tok, wt = idxs[e], wts[e]
        seg = yT[:, offs[e]:offs[e] + loads[e]].T  # [load, D]
        out[tok] += wt[:, None] * seg
    out = out.reshape(B, S, D)
    if _trace:
        return out, res
    return out


# revision 9
# speedup vs baseline: 1.0133x; 1.0133x over previous
"""Trainium2 Bass kernel for nn_BlockSparseMoE (top-2 of 8 experts, SwiGLU).

Strategy (8-way tensor-parallel over FFN):
  - Host: compute router (gate matmul + softmax + top-2 + renorm) in fp64,
    gather each expert's tokens into a contiguous column range of one
    shared xT matrix.
  - Device (SPMD x8): every core holds a 512-wide F-slice of ALL 8
    experts' w1/w3/w2 (same ~25 MB weight traffic as one full expert in
    the expert-parallel layout) and runs all 8192 token-expert pairs
    against its slice — exactly T*K/8 = 1024 pair-equivalents per core
    regardless of routing imbalance. Partial y outputs (transposed,
    unscaled) stream back.
  - Host: sum the 8 partial outputs, scale by the renormalized top-2
    weight, scatter-add per token.

Per-core layout:
  phase A: hT[f, t] = silu(x@w1)^T * (x@w3)^T per expert (FT=4 f-tiles of
           128), lhsT = w1 tile [128, 128f], rhs = xT d-chunk
           [128, tchunk] — weights stationary, tokens moving.
  phase B: yT[d, t] = w2_slice^T @ hT, lhsT = w2 f-tile [128f, 128d],
           rhs = hT f-tile [128, tchunk] — tokens moving, so ragged
           expert tails cost no extra PE cycles; no on-device scaling.

Startup: expert 0's weights are pre-tiled per f-tile ([128, DC, 128]
  each, 256 KB) and its first chunk is only 256 cols, so the first real
  matmul's inputs (~1 MB of DMA) land ~5us earlier than a monolithic
  load; a short HAM warmup bridges kernel entry to first-weights.
"""

import numpy as np
import ml_dtypes

HIDDEN = 1024
FFN = 4096
NUM_EXPERTS = 8
TOP_K = 2
N_CORES = 8
FS = FFN // N_CORES          # 512-wide F-slice per core
DC = HIDDEN // 128           # 8 contraction chunks for x@w1
FT = FS // 128               # 4 f-tiles per expert slice
DT = HIDDEN // 128           # 8 output d-tiles

_BF16 = ml_dtypes.bfloat16
_nc_cache = {}


# ---------------------------------------------------------------- router ----
def _route(x, gate_w, gate_b):
    """Top-2 routing. Returns per-expert (token_idx, renorm_weight)."""
    logits = x.astype(np.float64) @ gate_w.astype(np.float64) + gate_b.astype(
        np.float64
    )
    logits -= logits.max(axis=-1, keepdims=True)
    p = np.exp(logits)
    p /= p.sum(axis=-1, keepdims=True)
    # top-2 by prob, ties broken by lower index (matches jax.lax.top_k)
    top2 = np.argsort(-p, axis=-1, kind="stable")[:, :TOP_K]
    pt = np.take_along_axis(p, top2, axis=-1)
    wt = pt / pt.sum(axis=-1, keepdims=True)
    idxs, wts = [], []
    for e in range(NUM_EXPERTS):
        mask = top2 == e  # [T, 2]
        tok = np.nonzero(mask.any(axis=-1))[0]
        w = wt[tok, np.argmax(mask[tok], axis=-1)]
        idxs.append(tok)
        wts.append(w.astype(np.float32))
    return idxs, wts


def _chunks_for(load):
    """Split a token count into moving-dim chunks: all but the last are
    multiples of 128 in [256, 512]; keep the ragged tail >= 240 when
    possible (short moving dims go LDWEIGHTS-bound)."""
    C = load
    n = max(1, -(-C // 512))
    chunks = []
    rem = C
    for i in range(n - 1):
        c = min(512, -(-rem // ((n - i) * 128)) * 128)
        chunks.append(c)
        rem -= c
    while n > 1 and rem < 240 and chunks:
        for i in range(len(chunks)):
            if rem >= 240:
                break
            if chunks[i] > 256:
                chunks[i] -= 128
                rem += 128
        else:
            break
    chunks.append(rem)
    assert sum(chunks) == C and all(c > 0 for c in chunks)
    return tuple(chunks)


def _chunks_first(load):
    """Expert 0 gets a small 256-col first chunk so the first matmul's
    xT DMA is small; 256 is the LDWEIGHTS/matmul breakeven width."""
    if load >= 640:
        return (256,) + _chunks_for(load - 256)
    return _chunks_for(load)


def _plan(loads):
    """Per-expert chunk tuples + 128-aligned xT column offsets."""
    chunks_e, offs = [], []
    off = 0
    for e, l in enumerate(loads):
        chunks_e.append(_chunks_first(l) if e == 0 else _chunks_for(l))
        offs.append(off)
        off += -(-l // 128) * 128
    return tuple(chunks_e), tuple(offs), off


# ------------------------------------------------------------- device IR ----
def _build(plan):
    """Per-core Bacc graph. plan = (chunks_e, offs, XWT)."""
    import concourse.bacc as bacc
    import concourse.bass as bass
    import concourse.mybir as mybir
    import concourse.tile as tile

    chunks_e, offs, XWT = plan
    XW_e = [-(-sum(ch) // 128) * 128 for ch in chunks_e]

    bf16 = mybir.dt.bfloat16
    f32 = mybir.dt.float32

    nc = bacc.Bacc("TRN2", target_bir_lowering=False, debug=False,
                   num_devices=N_CORES)

    xT_d = nc.dram_tensor("xT", [HIDDEN, XWT], bf16, kind="ExternalInput")
    # w1s/w3s host-pre-tiled per expert as [e, p, ft, dc, 128]; w2s as
    # [e, p, ft, HIDDEN] so every DMA line is fully contiguous and any
    # single f-tile of an expert is one contiguous 2KB/partition run
    w1_d = nc.dram_tensor("w1s", [NUM_EXPERTS, 128, FT, DC, 128], bf16,
                          kind="ExternalInput")
    w3_d = nc.dram_tensor("w3s", [NUM_EXPERTS, 128, FT, DC, 128], bf16,
                          kind="ExternalInput")
    w2_d = nc.dram_tensor("w2s", [NUM_EXPERTS, 128, FT, HIDDEN], bf16,
                          kind="ExternalInput")
    y_d = nc.dram_tensor("y", [HIDDEN, XWT], bf16, kind="ExternalOutput")

    # boot blobs: the first f-tile chain's inputs packed per-partition
    # contiguous, so the whole load is 128 descriptors instead of ~1600
    # (the DMA queues are descriptor-rate-bound at startup).
    # boot1 = [w1-e0-ft0 (dc-major) | xT-e0 chunk0 (dc-major)], boot2 =
    # w3-e0-ft0.
    c0 = chunks_e[0][0]
    B1W = DC * 128 + DC * c0
    boot1_d = nc.dram_tensor("boot1", [128, B1W], bf16, kind="ExternalInput")
    boot2_d = nc.dram_tensor("boot2", [128, DC * 128], bf16,
                             kind="ExternalInput")

    xT_v = xT_d.ap().rearrange("(dc p) c -> p dc c", p=128)
    y_v = y_d.ap().rearrange("(dt p) c -> dt p c", p=128)

    with tile.TileContext(nc) as tc:
        with (
            tc.tile_pool(name="xe", bufs=2) as xep,
            tc.tile_pool(name="w13", bufs=2) as w13,
            tc.tile_pool(name="w2p", bufs=2) as w2p,
            tc.tile_pool(name="hp", bufs=2) as hp,
            tc.tile_pool(name="sil", bufs=4) as silp,
            tc.tile_pool(name="yo", bufs=8) as yop,
            tc.tile_pool(name="yq", bufs=4) as yqp,
            tc.tile_pool(name="ps", bufs=2, space=bass.MemorySpace.PSUM) as ps,
            tc.tile_pool(name="yps", bufs=4, space=bass.MemorySpace.PSUM) as yps,
        ):
            xe_tiles = {}
            w13_tiles = {}
            w2_tiles = {}

            def load_w13(e):
                w1_sb = w13.tile([128, FT, DC, 128], bf16, tag="w1",
                                 name="w1_sb")
                w3_sb = w13.tile([128, FT, DC, 128], bf16, tag="w3",
                                 name="w3_sb")
                nc.sync.dma_start(w1_sb[:], w1_d.ap()[e])
                nc.sync.dma_start(w3_sb[:], w3_d.ap()[e])
                w13_tiles[e] = (w1_sb, w3_sb)

            def load_xe(e, col0=0, col1=None):
                if e not in xe_tiles:
                    xe_tiles[e] = xep.tile([128, DC, XW_e[e]], bf16,
                                           tag="xT", name="xe_sb")
                if col1 is None:
                    col1 = XW_e[e]
                nc.sync.dma_start(
                    xe_tiles[e][:, :, col0:col1],
                    xT_v[:, :, offs[e] + col0:offs[e] + col1],
                )

            def load_w2(e):
                w2_sb = w2p.tile([128, FT, HIDDEN], bf16, tag="w2",
                                 name="w2_sb")
                nc.sync.dma_start(w2_sb[:], w2_d.ap()[e])
                w2_tiles[e] = w2_sb

            # ---- startup: boot blobs first (128 descriptors each), then
            # expert 0's remaining f-tiles one at a time, then w2/xe/rest.
            b1 = w13.tile([128, B1W], bf16, tag="b1", bufs=1, name="b1")
            b2 = w13.tile([128, DC * 128], bf16, tag="b2", bufs=1, name="b2")
            nc.sync.dma_start(b1[:], boot1_d.ap())
            nc.sync.dma_start(b2[:], boot2_d.ap())
            w1f = [None] + [w13.tile([128, DC, 128], bf16, tag=f"w1f{ft}",
                                     bufs=1, name=f"w1f{ft}")
                            for ft in range(1, FT)]
            w3f = [None] + [w13.tile([128, DC, 128], bf16, tag=f"w3f{ft}",
                                     bufs=1, name=f"w3f{ft}")
                            for ft in range(1, FT)]
            for ft in range(1, FT):
                nc.sync.dma_start(w1f[ft][:], w1_d.ap()[0][:, ft])
                nc.sync.dma_start(w3f[ft][:], w3_d.ap()[0][:, ft])
            load_w2(0)
            if XW_e[0] > c0:
                xe_tiles[0] = xep.tile([128, DC, XW_e[0]], bf16, tag="xT",
                                       name="xe_sb0")
                load_xe(0, col0=c0, col1=min(XW_e[0], c0 + 512))
            if XW_e[0] > c0 + 512:
                load_xe(0, col0=c0 + 512)

            def w1slice(e, ft, dc):
                if e == 0:
                    if ft == 0:
                        return b1[:, dc * 128:(dc + 1) * 128]
                    return w1f[ft][:, dc, :]
                return w13_tiles[e][0][:, ft, dc, :]

            def w3slice(e, ft, dc):
                if e == 0:
                    if ft == 0:
                        return b2[:, dc * 128:(dc + 1) * 128]
                    return w3f[ft][:, dc, :]
                return w13_tiles[e][1][:, ft, dc, :]

            def xslice(e, ci, dc, t0, chunk):
                if e == 0 and ci == 0:
                    base = DC * 128 + dc * c0
                    return b1[:, base + t0:base + t0 + chunk]
                return xe_tiles[e][:, dc, t0:t0 + chunk]

            # HAM pre-warm: keep the PE busy from kernel entry until the
            # boot blobs land (~11us) so the clock is fully ramped before
            # real work starts. memset on DVE (idle at start; gpsimd's
            # memset splits into 5 slices and delays the first LDW).
            warm_sb = silp.tile([128, 128], bf16, tag="warm_in", bufs=1)
            nc.vector.memset(warm_sb[:], 0.0)
            warm_ps = ps.tile([128, 128], f32, tag="ph1", name="warm_ps")
            N_WARM = 40
            for i in range(N_WARM):
                nc.tensor.matmul(warm_ps[:], warm_sb[:], warm_sb[:],
                                 start=(i == 0), stop=(i == N_WARM - 1))

            for e in range(NUM_EXPERTS):
                t0 = 0
                nch = len(chunks_e[e])
                # which chunk carries the next-expert prefetches: for e0
                # use the second chunk (chunk0 is tiny and its DMA window
                # is already packed with e0's own loads)
                pf_ci = (1 if nch > 1 else 0) if e == 0 else 0
                for ci, chunk in enumerate(chunks_e[e]):
                    hT = hp.tile([128, FT, chunk], bf16, tag="hT")
                    # ---- phase A ----
                    for ft in range(FT):
                        # prefetches ride behind this chunk's compute
                        if ci == pf_ci and e + 1 < NUM_EXPERTS:
                            if ft == 2:
                                load_w13(e + 1)
                            elif ft == 3:
                                load_xe(e + 1)
                                load_w2(e + 1)
                        ph1 = ps.tile([128, chunk], f32, tag="ph1")
                        ph3 = ps.tile([128, chunk], f32, tag="ph3")
                        for dc in range(DC):
                            nc.tensor.matmul(
                                ph1[:],
                                w1slice(e, ft, dc),
                                xslice(e, ci, dc, t0, chunk),
                                start=(dc == 0), stop=(dc == DC - 1),
                            )
                        for dc in range(DC):
                            nc.tensor.matmul(
                                ph3[:],
                                w3slice(e, ft, dc),
                                xslice(e, ci, dc, t0, chunk),
                                start=(dc == 0), stop=(dc == DC - 1),
                            )
                        sil = silp.tile([128, chunk], bf16, tag="sil")
                        nc.scalar.activation(
                            sil[:], ph1[:], mybir.ActivationFunctionType.Silu
                        )
                        nc.vector.tensor_mul(hT[:, ft, :], sil[:], ph3[:])

                    # ---- phase B: yT[d, t] partial, unscaled ----
                    w2_sb = w2_tiles[e]
                    is_last = (e == NUM_EXPERTS - 1
                               and ci == len(chunks_e[e]) - 1)
                    for dt in range(DT):
                        yp = yps.tile([128, chunk], f32, tag="yp")
                        for ft in range(FT):
                            nc.tensor.matmul(
                                yp[:],
                                w2_sb[:, ft, dt * 128:(dt + 1) * 128],
                                hT[:, ft, :],
                                start=(ft == 0), stop=(ft == FT - 1),
                            )
                        # alternate copies between ScalarE and DVE so
                        # neither engine becomes the bottleneck; on the
                        # very last output tile, drain in column-quarters
                        # on BOTH engines concurrently (separate tiles →
                        # no WAW serialization) so only a tiny copy+DMA
                        # trails the final matmul
                        col0 = offs[e] + t0
                        if is_last and dt == DT - 1:
                            q = -(-chunk // 4)
                            for p0 in range(0, chunk, q):
                                p1 = min(chunk, p0 + q)
                                ysb = yqp.tile([128, q], bf16, tag="yq")
                                if (p0 // q) % 2 == 0:
                                    nc.scalar.copy(ysb[:, 0:p1 - p0],
                                                   yp[:, p0:p1])
                                else:
                                    nc.vector.tensor_copy(ysb[:, 0:p1 - p0],
                                                          yp[:, p0:p1])
                                nc.sync.dma_start(
                                    y_v[dt][:, col0 + p0:col0 + p1],
                                    ysb[:, 0:p1 - p0],
                                )
                        else:
                            ysb = yop.tile([128, chunk], bf16, tag="ysb")
                            if dt % 2 == 0:
                                nc.scalar.copy(ysb[:], yp[:])
                            else:
                                nc.vector.tensor_copy(ysb[:], yp[:])
                            nc.sync.dma_start(
                                y_v[dt][:, col0:col0 + chunk],
                                ysb[:],
                            )
                    t0 += chunk
    nc.compile()
    return nc


def _get_nc(plan):
    if plan not in _nc_cache:
        _nc_cache[plan] = _build(plan)
    return _nc_cache[plan]


# ---------------------------------------------------------------- kernel ----
def kernel(hidden_states, gate_w, gate_b, w1, w3, w2, _trace=False):
    from concourse.bass_utils import run_bass_kernel_spmd

    B, S, D = hidden_states.shape
    T = B * S
    x = np.asarray(hidden_states, np.float32).reshape(T, D)
    idxs, wts = _route(x, np.asarray(gate_w, np.float32),
                       np.asarray(gate_b, np.float32))
    loads = [len(i) for i in idxs]
    plan = _plan(loads)
    chunks_e, offs, XWT = plan
    nc = _get_nc(plan)

    # shared xT: every expert's tokens in its 128-aligned column range
    xT = np.zeros((D, XWT), _BF16)
    for e in range(NUM_EXPERTS):
        xT[:, offs[e]:offs[e] + loads[e]] = x[idxs[e]].T.astype(_BF16)

    # weights pre-tiled for all cores in one reshape/transpose:
    # w1/w3 [E, D, F] -> [core, E, 128, ft, dc, 128]
    w1 = np.asarray(w1, np.float32)
    w3 = np.asarray(w3, np.float32)
    w2 = np.asarray(w2, np.float32)
    w1t = np.ascontiguousarray(
        w1.reshape(NUM_EXPERTS, DC, 128, N_CORES, FT, 128)
        .transpose(3, 0, 2, 4, 1, 5)).astype(_BF16)
    w3t = np.ascontiguousarray(
        w3.reshape(NUM_EXPERTS, DC, 128, N_CORES, FT, 128)
        .transpose(3, 0, 2, 4, 1, 5)).astype(_BF16)
    # w2 [E, F, D] -> [core, E, 128, ft, D]
    w2t = np.ascontiguousarray(
        w2.reshape(NUM_EXPERTS, N_CORES, FT, 128, HIDDEN)
        .transpose(1, 0, 3, 2, 4)).astype(_BF16)

    # boot blobs: per-partition-contiguous pack of [w1-e0-ft0 | xT chunk0]
    # and [w3-e0-ft0] (see _build)
    c0 = chunks_e[0][0]
    xb = np.ascontiguousarray(
        xT.reshape(DC, 128, XWT)[:, :, 0:c0]
        .transpose(1, 0, 2).reshape(128, DC * c0))
    boot1 = [np.concatenate(
        [w1t[c, 0, :, 0].reshape(128, DC * 128), xb], axis=1)
        for c in range(N_CORES)]
    boot2 = [np.ascontiguousarray(w3t[c, 0, :, 0].reshape(128, DC * 128))
             for c in range(N_CORES)]

    in_maps = [{
        "xT": xT,
        "w1s": w1t[c],
        "w3s": w3t[c],
        "w2s": w2t[c],
        "boot1": boot1[c],
        "boot2": boot2[c],
    } for c in range(N_CORES)]

    res = run_bass_kernel_spmd(nc, in_maps, core_ids=list(range(N_CORES)),
                               trace=_trace)

    yT = res.results[0]["y"].astype(np.float32)
    for c in range(1, N_CORES):
        yT += res.results[c]["y"].astype(np.float32)
    out = np.zeros((T, D), np.float32)
    for e in range(NUM_EXPERTS):
        tok, wt = idxs[e], wts[e]
        seg = yT[:, offs[e]:offs[e] + loads[e]].T  # [load, D]
        out[tok] += wt[:, None] * seg
    out = out.reshape(B, S, D)
    if _trace:
        return out, res
    return out


# revision 14
# speedup vs baseline: 1.0186x; 1.0053x over previous
"""Trainium2 Bass kernel for nn_BlockSparseMoE (top-2 of 8 experts, SwiGLU).

Strategy (8-way tensor-parallel over FFN):
  - Host: compute router (gate matmul + softmax + top-2 + renorm) in fp64,
    gather each expert's tokens into a contiguous column range of one
    shared xT matrix.
  - Device (SPMD x8): every core holds a 512-wide F-slice of ALL 8
    experts' w1/w3/w2 (same ~25 MB weight traffic as one full expert in
    the expert-parallel layout) and runs all 8192 token-expert pairs
    against its slice — exactly T*K/8 = 1024 pair-equivalents per core
    regardless of routing imbalance. Partial y outputs (transposed,
    unscaled) stream back.
  - Host: sum the 8 partial outputs, scale by the renormalized top-2
    weight, scatter-add per token.

Per-core layout:
  phase A: hT[f, t] = silu(x@w1)^T * (x@w3)^T per expert (FT=4 f-tiles of
           128), lhsT = w1 tile [128, 128f], rhs = xT d-chunk
           [128, tchunk] — weights stationary, tokens moving.
  phase B: yT[d, t] = w2_slice^T @ hT, lhsT = w2 f-tile [128f, 128d],
           rhs = hT f-tile [128, tchunk] — tokens moving, so ragged
           expert tails cost no extra PE cycles; no on-device scaling.

Startup: the DMA queues only start fetching descriptors ~8.6us into the
  kernel and ramp slowly, and every DMA into a 128-partition SBUF tile
  costs >=128 descriptors. So the first expert's entire working set is
  packed into ONE per-partition-contiguous "boot" DRAM blob, split into
  cascaded 128-descriptor sections sized so each lands just before the
  compute that needs it: [w1ft0|w3ft0|x-chunk0(256)] -> ft1 pair -> ft2
  pair -> ft3 pair -> w2 -> x cols 256:768 -> x rest. A short HAM warmup
  bridges kernel entry to the first section landing (~11.4us).

Expert order: processed by descending load (position 0 needs >=1008
  tokens for the 256/512 boot chunking to apply), except the expert with
  the smallest tail chunk goes last so the final drain is cheapest.
"""

import numpy as np
import ml_dtypes

HIDDEN = 1024
FFN = 4096
NUM_EXPERTS = 8
TOP_K = 2
N_CORES = 8
FS = FFN // N_CORES          # 512-wide F-slice per core
DC = HIDDEN // 128           # 8 contraction chunks for x@w1
FT = FS // 128               # 4 f-tiles per expert slice
DT = HIDDEN // 128           # 8 output d-tiles

_BF16 = ml_dtypes.bfloat16
_nc_cache = {}


# ---------------------------------------------------------------- router ----
def _route(x, gate_w, gate_b):
    """Top-2 routing. Returns per-expert (token_idx, renorm_weight)."""
    logits = x.astype(np.float64) @ gate_w.astype(np.float64) + gate_b.astype(
        np.float64
    )
    logits -= logits.max(axis=-1, keepdims=True)
    p = np.exp(logits)
    p /= p.sum(axis=-1, keepdims=True)
    # top-2 by prob, ties broken by lower index (matches jax.lax.top_k)
    top2 = np.argsort(-p, axis=-1, kind="stable")[:, :TOP_K]
    pt = np.take_along_axis(p, top2, axis=-1)
    wt = pt / pt.sum(axis=-1, keepdims=True)
    idxs, wts = [], []
    for e in range(NUM_EXPERTS):
        mask = top2 == e  # [T, 2]
        tok = np.nonzero(mask.any(axis=-1))[0]
        w = wt[tok, np.argmax(mask[tok], axis=-1)]
        idxs.append(tok)
        wts.append(w.astype(np.float32))
    return idxs, wts


def _chunks_for(load):
    """Split a token count into moving-dim chunks: all but the last are
    multiples of 128 in [256, 512]; keep the ragged tail >= 240 when
    possible (short moving dims go LDWEIGHTS-bound)."""
    C = load
    n = max(1, -(-C // 512))
    chunks = []
    rem = C
    for i in range(n - 1):
        c = min(512, -(-rem // ((n - i) * 128)) * 128)
        chunks.append(c)
        rem -= c
    while n > 1 and rem < 240 and chunks:
        for i in range(len(chunks)):
            if rem >= 240:
                break
            if chunks[i] > 256:
                chunks[i] -= 128
                rem += 128
        else:
            break
    chunks.append(rem)
    assert sum(chunks) == C and all(c > 0 for c in chunks)
    return tuple(chunks)


def _chunks_first(load):
    """Position-0 expert: small 256-col first chunk (smallest boot DMA
    that is not LDWEIGHTS-bound), then 512 (matches the x boot section
    split), then the rest."""
    if load >= 1008:
        return (256, 512) + _chunks_for(load - 768)
    if load >= 640:
        return (256,) + _chunks_for(load - 256)
    return _chunks_for(load)


def _plan(loads_pos):
    """Per-position chunk tuples + 128-aligned xT column offsets.
    loads_pos is already in processing order."""
    chunks_e, offs = [], []
    off = 0
    for pos, l in enumerate(loads_pos):
        chunks_e.append(_chunks_first(l) if pos == 0 else _chunks_for(l))
        offs.append(off)
        off += -(-l // 128) * 128
    return tuple(chunks_e), tuple(offs), off


# ------------------------------------------------------------- device IR ----
def _build(plan):
    """Per-core Bacc graph. plan = (chunks_e, offs, XWT)."""
    import concourse.bacc as bacc
    import concourse.bass as bass
    import concourse.mybir as mybir
    import concourse.tile as tile

    chunks_e, offs, XWT = plan
    XW_e = [-(-sum(ch) // 128) * 128 for ch in chunks_e]

    bf16 = mybir.dt.bfloat16
    f32 = mybir.dt.float32

    nc = bacc.Bacc("TRN2", target_bir_lowering=False, debug=False,
                   num_devices=N_CORES)

    xT_d = nc.dram_tensor("xT", [HIDDEN, XWT], bf16, kind="ExternalInput")
    # w1s/w3s host-pre-tiled per expert as [e, p, ft, dc, 128]; w2s as
    # [e, p, ft, HIDDEN] so every DMA line is fully contiguous
    w1_d = nc.dram_tensor("w1s", [NUM_EXPERTS, 128, FT, DC, 128], bf16,
                          kind="ExternalInput")
    w3_d = nc.dram_tensor("w3s", [NUM_EXPERTS, 128, FT, DC, 128], bf16,
                          kind="ExternalInput")
    w2_d = nc.dram_tensor("w2s", [NUM_EXPERTS, 128, FT, HIDDEN], bf16,
                          kind="ExternalInput")
    y_d = nc.dram_tensor("y", [HIDDEN, XWT], bf16, kind="ExternalOutput")

    # boot blob sections (bf16 elems per partition); see module docstring.
    # x sections: chunk0 rides in A; section F is exactly chunk1's range;
    # G covers the remaining chunks (each chunk fully inside one section).
    c0 = chunks_e[0][0]
    L0 = sum(chunks_e[0])
    xm = c0 + chunks_e[0][1] if len(chunks_e[0]) >= 2 else L0
    SEC = [2 * DC * 128 + DC * c0]          # A: w1ft0 | w3ft0 | x chunk0
    SEC += [2 * DC * 128] * (FT - 1)        # B,C,D: ft1..3 pairs
    SEC += [FT * HIDDEN]                    # E: w2
    if xm > c0:
        SEC += [DC * (xm - c0)]             # F: x cols [c0, xm)
    if L0 > xm:
        SEC += [DC * (L0 - xm)]             # G: x cols [xm, L0)
    BW = sum(SEC)
    boot_d = nc.dram_tensor("boot", [128, BW], bf16, kind="ExternalInput")

    xT_v = xT_d.ap().rearrange("(dc p) c -> p dc c", p=128)
    y_v = y_d.ap().rearrange("(dt p) c -> dt p c", p=128)

    with tile.TileContext(nc) as tc:
        with (
            tc.tile_pool(name="xe", bufs=2) as xep,
            tc.tile_pool(name="w13", bufs=2) as w13,
            tc.tile_pool(name="w2p", bufs=2) as w2p,
            tc.tile_pool(name="hp", bufs=2) as hp,
            tc.tile_pool(name="sil", bufs=4) as silp,
            tc.tile_pool(name="yo", bufs=8) as yop,
            tc.tile_pool(name="ps", bufs=2, space=bass.MemorySpace.PSUM) as ps,
            tc.tile_pool(name="yps", bufs=4, space=bass.MemorySpace.PSUM) as yps,
        ):
            xe_tiles = {}
            w13_tiles = {}
            w2_tiles = {}

            def load_w13(e):
                w1_sb = w13.tile([128, FT, DC, 128], bf16, tag="w1",
                                 name="w1_sb")
                w3_sb = w13.tile([128, FT, DC, 128], bf16, tag="w3",
                                 name="w3_sb")
                nc.sync.dma_start(w1_sb[:], w1_d.ap()[e])
                nc.sync.dma_start(w3_sb[:], w3_d.ap()[e])
                w13_tiles[e] = (w1_sb, w3_sb)

            def load_xe(e):
                xe_tiles[e] = xep.tile([128, DC, XW_e[e]], bf16,
                                       tag="xT", name="xe_sb")
                nc.sync.dma_start(
                    xe_tiles[e][:],
                    xT_v[:, :, offs[e]:offs[e] + XW_e[e]],
                )

            def load_w2(e):
                w2_sb = w2p.tile([128, FT, HIDDEN], bf16, tag="w2",
                                 name="w2_sb")
                nc.sync.dma_start(w2_sb[:], w2_d.ap()[e])
                w2_tiles[e] = w2_sb

            # ---- startup: cascaded boot sections, each one DMA of 128
            # descriptors, issued in the order compute consumes them.
            bts = []
            off_el = 0
            for si, w in enumerate(SEC):
                bt = w13.tile([128, w], bf16, tag=f"boot{si}", bufs=1,
                              name=f"boot{si}")
                nc.sync.dma_start(bt[:], boot_d.ap()[:, off_el:off_el + w])
                bts.append(bt)
                off_el += w

            def w1slice(e, ft, dc):
                if e == 0:
                    b = bts[ft]
                    return b[:, dc * 128:(dc + 1) * 128]
                return w13_tiles[e][0][:, ft, dc, :]

            def w3slice(e, ft, dc):
                if e == 0:
                    b = bts[ft]
                    base = DC * 128
                    return b[:, base + dc * 128:base + (dc + 1) * 128]
                return w13_tiles[e][1][:, ft, dc, :]

            def w2slice(e, ft, d0, d1):
                if e == 0:
                    return bts[FT][:, ft * HIDDEN + d0:ft * HIDDEN + d1]
                return w2_tiles[e][:, ft, d0:d1]

            def xslice(e, dc, t0, chunk):
                if e == 0:
                    if t0 < c0:
                        assert t0 + chunk <= c0
                        base = 2 * DC * 128 + dc * c0
                        return bts[0][:, base + t0:base + t0 + chunk]
                    if t0 < xm:
                        assert t0 + chunk <= xm
                        b = bts[FT + 1]
                        base = dc * (xm - c0) + (t0 - c0)
                        return b[:, base:base + chunk]
                    b = bts[FT + 2]
                    base = dc * (L0 - xm) + (t0 - xm)
                    return b[:, base:base + chunk]
                return xe_tiles[e][:, dc, t0:t0 + chunk]

            # HAM pre-warm: keep the PE busy from kernel entry (~7.6us)
            # until boot section A lands (~11.4us) so the clock is fully
            # ramped when real work starts.
            warm_sb = silp.tile([128, 128], bf16, tag="warm_in", bufs=1)
            nc.vector.memset(warm_sb[:], 0.0)
            warm_ps = ps.tile([128, 128], f32, tag="ph1", name="warm_ps")
            N_WARM = 35
            for i in range(N_WARM):
                nc.tensor.matmul(warm_ps[:], warm_sb[:], warm_sb[:],
                                 start=(i == 0), stop=(i == N_WARM - 1))

            for e in range(NUM_EXPERTS):
                t0 = 0
                nch = len(chunks_e[e])
                # which chunk carries the next-expert prefetches: for e0
                # use the second chunk (chunk0's DMA window is packed
                # with the boot cascade)
                pf_ci = (1 if nch > 1 else 0) if e == 0 else 0
                for ci, chunk in enumerate(chunks_e[e]):
                    hT = hp.tile([128, FT, chunk], bf16, tag="hT")
                    # ---- phase A ----
                    for ft in range(FT):
                        if ci == pf_ci and e + 1 < NUM_EXPERTS:
                            if ft == 2:
                                load_w13(e + 1)
                            elif ft == 3:
                                load_xe(e + 1)
                                load_w2(e + 1)
                        ph1 = ps.tile([128, chunk], f32, tag="ph1")
                        ph3 = ps.tile([128, chunk], f32, tag="ph3")
                        for dc in range(DC):
                            nc.tensor.matmul(
                                ph1[:],
                                w1slice(e, ft, dc),
                                xslice(e, dc, t0, chunk),
                                start=(dc == 0), stop=(dc == DC - 1),
                            )
                        for dc in range(DC):
                            nc.tensor.matmul(
                                ph3[:],
                                w3slice(e, ft, dc),
                                xslice(e, dc, t0, chunk),
                                start=(dc == 0), stop=(dc == DC - 1),
                            )
                        sil = silp.tile([128, chunk], bf16, tag="sil")
                        nc.scalar.activation(
                            sil[:], ph1[:], mybir.ActivationFunctionType.Silu
                        )
                        nc.vector.tensor_mul(hT[:, ft, :], sil[:], ph3[:])

                    # ---- phase B: yT[d, t] partial, unscaled ----
                    for dt in range(DT):
                        yp = yps.tile([128, chunk], f32, tag="yp")
                        for ft in range(FT):
                            nc.tensor.matmul(
                                yp[:],
                                w2slice(e, ft, dt * 128, (dt + 1) * 128),
                                hT[:, ft, :],
                                start=(ft == 0), stop=(ft == FT - 1),
                            )
                        # alternate copies between ScalarE and DVE so
                        # neither engine becomes the bottleneck (PSUM
                        # same-bank reads serialize, so no quartering)
                        ysb = yop.tile([128, chunk], bf16, tag="ysb")
                        if dt % 2 == 0:
                            nc.scalar.copy(ysb[:], yp[:])
                        else:
                            nc.vector.tensor_copy(ysb[:], yp[:])
                        col0 = offs[e] + t0
                        nc.sync.dma_start(
                            y_v[dt][:, col0:col0 + chunk],
                            ysb[:],
                        )
                    t0 += chunk
    nc.compile()
    return nc


def _get_nc(plan):
    if plan not in _nc_cache:
        _nc_cache[plan] = _build(plan)
    return _nc_cache[plan]


# ---------------------------------------------------------------- kernel ----
def kernel(hidden_states, gate_w, gate_b, w1, w3, w2, _trace=False):
    from concourse.bass_utils import run_bass_kernel_spmd

    B, S, D = hidden_states.shape
    T = B * S
    x = np.asarray(hidden_states, np.float32).reshape(T, D)
    idxs, wts = _route(x, np.asarray(gate_w, np.float32),
                       np.asarray(gate_b, np.float32))
    loads = [len(i) for i in idxs]

    # processing order: largest load first (boot chunking wants >=1008),
    # smallest tail chunk last (cheapest final drain)
    order = sorted(range(NUM_EXPERTS), key=lambda e: -loads[e])
    tail = {e: _chunks_for(loads[e])[-1] for e in order[1:]}
    last = min(order[1:], key=lambda e: tail[e])
    perm = [order[0]] + [e for e in order[1:] if e != last] + [last]

    idxs = [idxs[e] for e in perm]
    wts = [wts[e] for e in perm]
    loads = [loads[e] for e in perm]
    plan = _plan(loads)
    chunks_e, offs, XWT = plan
    nc = _get_nc(plan)

    # shared xT: every expert's tokens in its 128-aligned column range,
    # in processing order
    xT = np.zeros((D, XWT), _BF16)
    for e in range(NUM_EXPERTS):
        xT[:, offs[e]:offs[e] + loads[e]] = x[idxs[e]].T.astype(_BF16)

    # weights pre-tiled for all cores in one reshape/transpose, expert
    # axis permuted into processing order:
    # w1/w3 [E, D, F] -> [core, E, 128, ft, dc, 128]
    w1 = np.asarray(w1, np.float32)[perm]
    w3 = np.asarray(w3, np.float32)[perm]
    w2 = np.asarray(w2, np.float32)[perm]
    w1t = np.ascontiguousarray(
        w1.reshape(NUM_EXPERTS, DC, 128, N_CORES, FT, 128)
        .transpose(3, 0, 2, 4, 1, 5)).astype(_BF16)
    w3t = np.ascontiguousarray(
        w3.reshape(NUM_EXPERTS, DC, 128, N_CORES, FT, 128)
        .transpose(3, 0, 2, 4, 1, 5)).astype(_BF16)
    # w2 [E, F, D] -> [core, E, 128, ft, D]
    w2t = np.ascontiguousarray(
        w2.reshape(NUM_EXPERTS, N_CORES, FT, 128, HIDDEN)
        .transpose(1, 0, 3, 2, 4)).astype(_BF16)

    # boot blob (see _build): per-partition-contiguous pack of expert
    # pos-0's whole working set in consumption order
    c0 = chunks_e[0][0]
    L0 = sum(chunks_e[0])
    xm = c0 + chunks_e[0][1] if len(chunks_e[0]) >= 2 else L0

    def xsec(a, b):
        return np.ascontiguousarray(
            xT.reshape(DC, 128, XWT)[:, :, a:b]
            .transpose(1, 0, 2).reshape(128, DC * (b - a)))

    boots = []
    for c in range(N_CORES):
        secs = [w1t[c, 0, :, 0].reshape(128, DC * 128),
                w3t[c, 0, :, 0].reshape(128, DC * 128),
                xsec(0, c0)]
        for ft in range(1, FT):
            secs += [w1t[c, 0, :, ft].reshape(128, DC * 128),
                     w3t[c, 0, :, ft].reshape(128, DC * 128)]
        secs += [w2t[c, 0].reshape(128, FT * HIDDEN)]
        if xm > c0:
            secs += [xsec(c0, xm)]
        if L0 > xm:
            secs += [xsec(xm, L0)]
        boots.append(np.concatenate(secs, axis=1))

    in_maps = [{
        "xT": xT,
        "w1s": w1t[c],
        "w3s": w3t[c],
        "w2s": w2t[c],
        "boot": boots[c],
    } for c in range(N_CORES)]

    res = run_bass_kernel_spmd(nc, in_maps, core_ids=list(range(N_CORES)),
                               trace=_trace)

    yT = res.results[0]["y"].astype(np.float32)
    for c in range(1, N_CORES):
        yT += res.results[c]["y"].astype(np.float32)
    out = np.zeros((T, D), np.float32)
    for e in range(NUM_EXPERTS):
        tok, wt = idxs[e], wts[e]
        seg = yT[:, offs[e]:offs[e] + loads[e]].T  # [load, D]
        out[tok] += wt[:, None] * seg
    out = out.reshape(B, S, D)
    if _trace:
        return out, res
    return out


# revision 20
# speedup vs baseline: 1.0215x; 1.0028x over previous
"""Trainium2 Bass kernel for nn_BlockSparseMoE (top-2 of 8 experts, SwiGLU).

Strategy (8-way tensor-parallel over FFN):
  - Host: compute router (gate matmul + softmax + top-2 + renorm) in fp64,
    gather each expert's tokens into a contiguous column range of one
    shared xT matrix.
  - Device (SPMD x8): every core holds a 512-wide F-slice of ALL 8
    experts' w1/w3/w2 (same ~25 MB weight traffic as one full expert in
    the expert-parallel layout) and runs all 8192 token-expert pairs
    against its slice — exactly T*K/8 = 1024 pair-equivalents per core
    regardless of routing imbalance. Partial y outputs (transposed,
    unscaled) stream back.
  - Host: sum the 8 partial outputs, scale by the renormalized top-2
    weight, scatter-add per token.

Per-core layout:
  phase A: hT[f, t] = silu(x@w1)^T * (x@w3)^T per expert (FT=4 f-tiles of
           128), lhsT = w1 tile [128, 128f], rhs = xT d-chunk
           [128, tchunk] — weights stationary, tokens moving.
  phase B: yT[d, t] = w2_slice^T @ hT, lhsT = w2 f-tile [128f, 128d],
           rhs = hT f-tile [128, tchunk] — tokens moving, so ragged
           expert tails cost no extra PE cycles; no on-device scaling.

Startup: the DMA queues only start fetching descriptors ~8.6us into the
  kernel and ramp slowly, and every DMA into a 128-partition SBUF tile
  costs >=128 descriptors. So the first expert's entire working set is
  packed into ONE per-partition-contiguous "boot" DRAM blob, split into
  cascaded 128-descriptor sections sized so each lands just before the
  compute that needs it: [w1ft0|w3ft0|x-chunk0(256)] -> ft1 pair -> ft2
  pair -> ft3 pair -> w2 -> x cols 256:768 -> x rest. A short HAM warmup
  bridges kernel entry to the first section landing (~11.4us).

Expert order: processed by descending load (position 0 needs >=1008
  tokens for the 256/512 boot chunking to apply), except the expert with
  the smallest tail chunk goes last so the final drain is cheapest.
"""

import numpy as np
import ml_dtypes

HIDDEN = 1024
FFN = 4096
NUM_EXPERTS = 8
TOP_K = 2
N_CORES = 8
FS = FFN // N_CORES          # 512-wide F-slice per core
DC = HIDDEN // 128           # 8 contraction chunks for x@w1
FT = FS // 128               # 4 f-tiles per expert slice
DT = HIDDEN // 128           # 8 output d-tiles

_BF16 = ml_dtypes.bfloat16
_nc_cache = {}


# ---------------------------------------------------------------- router ----
def _route(x, gate_w, gate_b):
    """Top-2 routing. Returns per-expert (token_idx, renorm_weight)."""
    logits = x.astype(np.float64) @ gate_w.astype(np.float64) + gate_b.astype(
        np.float64
    )
    logits -= logits.max(axis=-1, keepdims=True)
    p = np.exp(logits)
    p /= p.sum(axis=-1, keepdims=True)
    # top-2 by prob, ties broken by lower index (matches jax.lax.top_k)
    top2 = np.argsort(-p, axis=-1, kind="stable")[:, :TOP_K]
    pt = np.take_along_axis(p, top2, axis=-1)
    wt = pt / pt.sum(axis=-1, keepdims=True)
    idxs, wts = [], []
    for e in range(NUM_EXPERTS):
        mask = top2 == e  # [T, 2]
        tok = np.nonzero(mask.any(axis=-1))[0]
        w = wt[tok, np.argmax(mask[tok], axis=-1)]
        idxs.append(tok)
        wts.append(w.astype(np.float32))
    return idxs, wts


def _chunks_for(load):
    """Split a token count into moving-dim chunks: all but the last are
    multiples of 128 in [256, 512]; keep the ragged tail >= 240 when
    possible (short moving dims go LDWEIGHTS-bound)."""
    C = load
    n = max(1, -(-C // 512))
    chunks = []
    rem = C
    for i in range(n - 1):
        c = min(512, -(-rem // ((n - i) * 128)) * 128)
        chunks.append(c)
        rem -= c
    while n > 1 and rem < 240 and chunks:
        for i in range(len(chunks)):
            if rem >= 240:
                break
            if chunks[i] > 256:
                chunks[i] -= 128
                rem += 128
        else:
            break
    chunks.append(rem)
    assert sum(chunks) == C and all(c > 0 for c in chunks)
    return tuple(chunks)


def _chunks_first(load):
    """Position-0 expert: small 256-col first chunk (smallest boot DMA
    that is not LDWEIGHTS-bound), then 512 (matches the x boot section
    split), then the rest."""
    if load >= 1008:
        return (256, 512) + _chunks_for(load - 768)
    if load >= 640:
        return (256,) + _chunks_for(load - 256)
    return _chunks_for(load)


def _plan(loads_pos):
    """Per-position chunk tuples + 128-aligned xT column offsets.
    loads_pos is already in processing order."""
    chunks_e, offs = [], []
    off = 0
    for pos, l in enumerate(loads_pos):
        chunks_e.append(_chunks_first(l) if pos == 0 else _chunks_for(l))
        offs.append(off)
        off += -(-l // 128) * 128
    return tuple(chunks_e), tuple(offs), off


# ------------------------------------------------------------- device IR ----
def _build(plan):
    """Per-core Bacc graph. plan = (chunks_e, offs, XWT)."""
    import concourse.bacc as bacc
    import concourse.bass as bass
    import concourse.mybir as mybir
    import concourse.tile as tile

    chunks_e, offs, XWT = plan
    XW_e = [-(-sum(ch) // 128) * 128 for ch in chunks_e]

    bf16 = mybir.dt.bfloat16
    f32 = mybir.dt.float32

    nc = bacc.Bacc("TRN2", target_bir_lowering=False, debug=False,
                   num_devices=N_CORES)

    xT_d = nc.dram_tensor("xT", [HIDDEN, XWT], bf16, kind="ExternalInput")
    # w1s/w3s host-pre-tiled per expert as [e, p, ft, dc, 128]; w2s as
    # [e, p, ft, HIDDEN] so every DMA line is fully contiguous
    w1_d = nc.dram_tensor("w1s", [NUM_EXPERTS, 128, FT, DC, 128], bf16,
                          kind="ExternalInput")
    w3_d = nc.dram_tensor("w3s", [NUM_EXPERTS, 128, FT, DC, 128], bf16,
                          kind="ExternalInput")
    w2_d = nc.dram_tensor("w2s", [NUM_EXPERTS, 128, FT, HIDDEN], bf16,
                          kind="ExternalInput")
    # y layout: per-chunk blocks, each [128, DT, chunk] per-partition
    # contiguous, so a 4-dt group DMA is 128 descriptors of 4*chunk*2B
    # (the flat [dt, p, chunk] layout needs 128 descs per dt => the last
    # chunk's 1024 output descriptors trail the final matmul by ~3us)
    y_d = nc.dram_tensor("y", [128, DT * XWT], bf16, kind="ExternalOutput")

    # boot blob sections (bf16 elems per partition); see module docstring.
    # x sections: chunk0 rides in A; section F is exactly chunk1's range;
    # G covers the remaining chunks (each chunk fully inside one section).
    c0 = chunks_e[0][0]
    L0 = sum(chunks_e[0])
    xm = c0 + chunks_e[0][1] if len(chunks_e[0]) >= 2 else L0
    SEC = [2 * DC * 128 + DC * c0]          # A: w1ft0 | w3ft0 | x chunk0
    SEC += [2 * DC * 128] * (FT - 1)        # B,C,D: ft1..3 pairs
    SEC += [FT * HIDDEN]                    # E: w2
    if xm > c0:
        SEC += [DC * (xm - c0)]             # F: x cols [c0, xm)
    if L0 > xm:
        SEC += [DC * (L0 - xm)]             # G: x cols [xm, L0)
    BW = sum(SEC)
    boot_d = nc.dram_tensor("boot", [128, BW], bf16, kind="ExternalInput")

    xT_v = xT_d.ap().rearrange("(dc p) c -> p dc c", p=128)

    with tile.TileContext(nc) as tc:
        with (
            tc.tile_pool(name="xe", bufs=2) as xep,
            tc.tile_pool(name="w13", bufs=2) as w13,
            tc.tile_pool(name="w2p", bufs=2) as w2p,
            tc.tile_pool(name="hp", bufs=2) as hp,
            tc.tile_pool(name="sil", bufs=4) as silp,
            tc.tile_pool(name="yo", bufs=8) as yop,
            tc.tile_pool(name="ps", bufs=2, space=bass.MemorySpace.PSUM) as ps,
            tc.tile_pool(name="yps", bufs=4, space=bass.MemorySpace.PSUM) as yps,
        ):
            xe_tiles = {}
            w13_tiles = {}
            w2_tiles = {}

            def load_w13(e):
                w1_sb = w13.tile([128, FT, DC, 128], bf16, tag="w1",
                                 name="w1_sb")
                w3_sb = w13.tile([128, FT, DC, 128], bf16, tag="w3",
                                 name="w3_sb")
                nc.sync.dma_start(w1_sb[:], w1_d.ap()[e])
                nc.sync.dma_start(w3_sb[:], w3_d.ap()[e])
                w13_tiles[e] = (w1_sb, w3_sb)

            def load_xe(e):
                xe_tiles[e] = xep.tile([128, DC, XW_e[e]], bf16,
                                       tag="xT", name="xe_sb")
                nc.sync.dma_start(
                    xe_tiles[e][:],
                    xT_v[:, :, offs[e]:offs[e] + XW_e[e]],
                )

            def load_w2(e):
                w2_sb = w2p.tile([128, FT, HIDDEN], bf16, tag="w2",
                                 name="w2_sb")
                nc.sync.dma_start(w2_sb[:], w2_d.ap()[e])
                w2_tiles[e] = w2_sb

            # ---- startup: cascaded boot sections, each one DMA of 128
            # descriptors, issued in the order compute consumes them and
            # alternated across both HWDGE rings (SP and Act) so the
            # ~650ns-per-trigger DIRECT2D generation doesn't serialize
            # the cascade.
            bts = []
            off_el = 0
            for si, w in enumerate(SEC):
                bt = w13.tile([128, w], bf16, tag=f"boot{si}", bufs=1,
                              name=f"boot{si}")
                eng = nc.sync if si % 2 == 0 else nc.scalar
                eng.dma_start(bt[:], boot_d.ap()[:, off_el:off_el + w])
                bts.append(bt)
                off_el += w

            def w1slice(e, ft, dc):
                if e == 0:
                    b = bts[ft]
                    return b[:, dc * 128:(dc + 1) * 128]
                return w13_tiles[e][0][:, ft, dc, :]

            def w3slice(e, ft, dc):
                if e == 0:
                    b = bts[ft]
                    base = DC * 128
                    return b[:, base + dc * 128:base + (dc + 1) * 128]
                return w13_tiles[e][1][:, ft, dc, :]

            def w2slice(e, ft, d0, d1):
                if e == 0:
                    return bts[FT][:, ft * HIDDEN + d0:ft * HIDDEN + d1]
                return w2_tiles[e][:, ft, d0:d1]

            def xslice(e, dc, t0, chunk):
                if e == 0:
                    if t0 < c0:
                        assert t0 + chunk <= c0
                        base = 2 * DC * 128 + dc * c0
                        return bts[0][:, base + t0:base + t0 + chunk]
                    if t0 < xm:
                        assert t0 + chunk <= xm
                        b = bts[FT + 1]
                        base = dc * (xm - c0) + (t0 - c0)
                        return b[:, base:base + chunk]
                    b = bts[FT + 2]
                    base = dc * (L0 - xm) + (t0 - xm)
                    return b[:, base:base + chunk]
                return xe_tiles[e][:, dc, t0:t0 + chunk]

            # HAM pre-warm: keep the PE busy from kernel entry (~7.6us)
            # until boot section A lands (~11.4us) so the clock is fully
            # ramped when real work starts.
            warm_sb = silp.tile([128, 128], bf16, tag="warm_in", bufs=1)
            nc.vector.memset(warm_sb[:], 0.0)
            warm_ps = ps.tile([128, 128], f32, tag="ph1", name="warm_ps")
            N_WARM = 35
            for i in range(N_WARM):
                nc.tensor.matmul(warm_ps[:], warm_sb[:], warm_sb[:],
                                 start=(i == 0), stop=(i == N_WARM - 1))

            cum_cols = 0
            for e in range(NUM_EXPERTS):
                t0 = 0
                nch = len(chunks_e[e])
                # which chunk carries the next-expert prefetches: for e0
                # use the second chunk (chunk0's DMA window is packed
                # with the boot cascade)
                pf_ci = (1 if nch > 1 else 0) if e == 0 else 0
                for ci, chunk in enumerate(chunks_e[e]):
                    hT = hp.tile([128, FT, chunk], bf16, tag="hT")
                    # ---- phase A ----
                    for ft in range(FT):
                        if ci == pf_ci and e + 1 < NUM_EXPERTS:
                            if ft == 2:
                                load_w13(e + 1)
                            elif ft == 3:
                                load_xe(e + 1)
                                load_w2(e + 1)
                        ph1 = ps.tile([128, chunk], f32, tag="ph1")
                        ph3 = ps.tile([128, chunk], f32, tag="ph3")
                        for dc in range(DC):
                            nc.tensor.matmul(
                                ph1[:],
                                w1slice(e, ft, dc),
                                xslice(e, dc, t0, chunk),
                                start=(dc == 0), stop=(dc == DC - 1),
                            )
                        for dc in range(DC):
                            nc.tensor.matmul(
                                ph3[:],
                                w3slice(e, ft, dc),
                                xslice(e, dc, t0, chunk),
                                start=(dc == 0), stop=(dc == DC - 1),
                            )
                        sil = silp.tile([128, chunk], bf16, tag="sil")
                        nc.scalar.activation(
                            sil[:], ph1[:], mybir.ActivationFunctionType.Silu
                        )
                        nc.vector.tensor_mul(hT[:, ft, :], sil[:], ph3[:])

                    # ---- phase B: yT[d, t] partial, unscaled ----
                    ybase = DT * cum_cols
                    ysb = None
                    for dt in range(DT):
                        if dt % 4 == 0:
                            ysb = yop.tile([128, 4, chunk], bf16, tag="ysb")
                        yp = yps.tile([128, chunk], f32, tag="yp")
                        for ft in range(FT):
                            nc.tensor.matmul(
                                yp[:],
                                w2slice(e, ft, dt * 128, (dt + 1) * 128),
                                hT[:, ft, :],
                                start=(ft == 0), stop=(ft == FT - 1),
                            )
                        # alternate copies between ScalarE and DVE so
                        # neither engine becomes the bottleneck (PSUM
                        # same-bank reads serialize, so no quartering)
                        if dt % 2 == 0:
                            nc.scalar.copy(ysb[:, dt % 4, :], yp[:])
                        else:
                            nc.vector.tensor_copy(ysb[:, dt % 4, :], yp[:])
                        if dt % 4 == 3:
                            g0 = ybase + (dt - 3) * chunk
                            nc.sync.dma_start(
                                y_d.ap()[:, g0:g0 + 4 * chunk]
                                .rearrange("p (j c) -> p j c", j=4),
                                ysb[:],
                            )
                    t0 += chunk
                    cum_cols += chunk
    nc.compile()
    return nc


def _get_nc(plan):
    if plan not in _nc_cache:
        _nc_cache[plan] = _build(plan)
    return _nc_cache[plan]


# ---------------------------------------------------------------- kernel ----
def kernel(hidden_states, gate_w, gate_b, w1, w3, w2, _trace=False):
    from concourse.bass_utils import run_bass_kernel_spmd

    B, S, D = hidden_states.shape
    T = B * S
    x = np.asarray(hidden_states, np.float32).reshape(T, D)
    idxs, wts = _route(x, np.asarray(gate_w, np.float32),
                       np.asarray(gate_b, np.float32))
    loads = [len(i) for i in idxs]

    # processing order: largest load first (boot chunking wants >=1008),
    # smallest tail chunk last (cheapest final drain)
    order = sorted(range(NUM_EXPERTS), key=lambda e: -loads[e])
    tail = {e: _chunks_for(loads[e])[-1] for e in order[1:]}
    last = min(order[1:], key=lambda e: tail[e])
    perm = [order[0]] + [e for e in order[1:] if e != last] + [last]

    idxs = [idxs[e] for e in perm]
    wts = [wts[e] for e in perm]
    loads = [loads[e] for e in perm]
    plan = _plan(loads)
    chunks_e, offs, XWT = plan
    nc = _get_nc(plan)

    # shared xT: every expert's tokens in its 128-aligned column range,
    # in processing order
    xT = np.zeros((D, XWT), _BF16)
    for e in range(NUM_EXPERTS):
        xT[:, offs[e]:offs[e] + loads[e]] = x[idxs[e]].T.astype(_BF16)

    # weights pre-tiled for all cores in one reshape/transpose, expert
    # axis permuted into processing order:
    # w1/w3 [E, D, F] -> [core, E, 128, ft, dc, 128]
    w1 = np.asarray(w1, np.float32)[perm]
    w3 = np.asarray(w3, np.float32)[perm]
    w2 = np.asarray(w2, np.float32)[perm]
    w1t = np.ascontiguousarray(
        w1.reshape(NUM_EXPERTS, DC, 128, N_CORES, FT, 128)
        .transpose(3, 0, 2, 4, 1, 5)).astype(_BF16)
    w3t = np.ascontiguousarray(
        w3.reshape(NUM_EXPERTS, DC, 128, N_CORES, FT, 128)
        .transpose(3, 0, 2, 4, 1, 5)).astype(_BF16)
    # w2 [E, F, D] -> [core, E, 128, ft, D]
    w2t = np.ascontiguousarray(
        w2.reshape(NUM_EXPERTS, N_CORES, FT, 128, HIDDEN)
        .transpose(1, 0, 3, 2, 4)).astype(_BF16)

    # boot blob (see _build): per-partition-contiguous pack of expert
    # pos-0's whole working set in consumption order
    c0 = chunks_e[0][0]
    L0 = sum(chunks_e[0])
    xm = c0 + chunks_e[0][1] if len(chunks_e[0]) >= 2 else L0

    def xsec(a, b):
        return np.ascontiguousarray(
            xT.reshape(DC, 128, XWT)[:, :, a:b]
            .transpose(1, 0, 2).reshape(128, DC * (b - a)))

    boots = []
    for c in range(N_CORES):
        secs = [w1t[c, 0, :, 0].reshape(128, DC * 128),
                w3t[c, 0, :, 0].reshape(128, DC * 128),
                xsec(0, c0)]
        for ft in range(1, FT):
            secs += [w1t[c, 0, :, ft].reshape(128, DC * 128),
                     w3t[c, 0, :, ft].reshape(128, DC * 128)]
        secs += [w2t[c, 0].reshape(128, FT * HIDDEN)]
        if xm > c0:
            secs += [xsec(c0, xm)]
        if L0 > xm:
            secs += [xsec(xm, L0)]
        boots.append(np.concatenate(secs, axis=1))

    in_maps = [{
        "xT": xT,
        "w1s": w1t[c],
        "w3s": w3t[c],
        "w2s": w2t[c],
        "boot": boots[c],
    } for c in range(N_CORES)]

    res = run_bass_kernel_spmd(nc, in_maps, core_ids=list(range(N_CORES)),
                               trace=_trace)

    y2 = res.results[0]["y"].astype(np.float32)
    for c in range(1, N_CORES):
        y2 += res.results[c]["y"].astype(np.float32)
    # y2 is per-chunk blocks [128, DT, chunk]; rebuild yT [HIDDEN, XWT]
    yT = np.zeros((D, XWT), np.float32)
    cum = 0
    for e in range(NUM_EXPERTS):
        t0 = 0
        for chunk in chunks_e[e]:
            blk = y2[:, DT * cum:DT * (cum + chunk)].reshape(128, DT, chunk)
            yT[:, offs[e] + t0:offs[e] + t0 + chunk] = (
                blk.transpose(1, 0, 2).reshape(D, chunk))
            t0 += chunk
            cum += chunk
    out = np.zeros((T, D), np.float32)
    for e in range(NUM_EXPERTS):
        tok, wt = idxs[e], wts[e]
        seg = yT[:, offs[e]:offs[e] + loads[e]].T  # [load, D]
        out[tok] += wt[:, None] * seg
    out = out.reshape(B, S, D)
    if _trace:
        return out, res
    return out


# revision 23
# speedup vs baseline: 1.0238x; 1.0022x over previous
"""Trainium2 Bass kernel for nn_BlockSparseMoE (top-2 of 8 experts, SwiGLU).

Strategy (8-way tensor-parallel over FFN):
  - Host: compute router (gate matmul + softmax + top-2 + renorm) in fp64,
    gather each expert's tokens into a contiguous column range of one
    shared xT matrix.
  - Device (SPMD x8): every core holds a 512-wide F-slice of ALL 8
    experts' w1/w3/w2 (same ~25 MB weight traffic as one full expert in
    the expert-parallel layout) and runs all 8192 token-expert pairs
    against its slice — exactly T*K/8 = 1024 pair-equivalents per core
    regardless of routing imbalance. Partial y outputs (transposed,
    unscaled) stream back.
  - Host: sum the 8 partial outputs, scale by the renormalized top-2
    weight, scatter-add per token.

Per-core layout:
  phase A: hT[f, t] = silu(x@w1)^T * (x@w3)^T per expert (FT=4 f-tiles of
           128), lhsT = w1 tile [128, 128f], rhs = xT d-chunk
           [128, tchunk] — weights stationary, tokens moving.
  phase B: yT[d, t] = w2_slice^T @ hT, lhsT = w2 f-tile [128f, 128d],
           rhs = hT f-tile [128, tchunk] — tokens moving, so ragged
           expert tails cost no extra PE cycles; no on-device scaling.

Startup: the DMA queues only start fetching descriptors ~8.6us into the
  kernel and ramp slowly, and every DMA into a 128-partition SBUF tile
  costs >=128 descriptors. So the first expert's entire working set is
  packed into ONE per-partition-contiguous "boot" DRAM blob, split into
  cascaded 128-descriptor sections sized so each lands just before the
  compute that needs it: [w1ft0|w3ft0|x-chunk0(256)] -> ft1 pair -> ft2
  pair -> ft3 pair -> w2 -> x cols 256:768 -> x rest. A short HAM warmup
  bridges kernel entry to the first section landing (~11.4us).

Expert order: processed by descending load (position 0 needs >=1008
  tokens for the 256/512 boot chunking to apply), except the expert with
  the smallest tail chunk goes last so the final drain is cheapest.
"""

import numpy as np
import ml_dtypes

HIDDEN = 1024
FFN = 4096
NUM_EXPERTS = 8
TOP_K = 2
N_CORES = 8
FS = FFN // N_CORES          # 512-wide F-slice per core
DC = HIDDEN // 128           # 8 contraction chunks for x@w1
FT = FS // 128               # 4 f-tiles per expert slice
DT = HIDDEN // 128           # 8 output d-tiles

_BF16 = ml_dtypes.bfloat16
_nc_cache = {}


# ---------------------------------------------------------------- router ----
def _route(x, gate_w, gate_b):
    """Top-2 routing. Returns per-expert (token_idx, renorm_weight)."""
    logits = x.astype(np.float64) @ gate_w.astype(np.float64) + gate_b.astype(
        np.float64
    )
    logits -= logits.max(axis=-1, keepdims=True)
    p = np.exp(logits)
    p /= p.sum(axis=-1, keepdims=True)
    # top-2 by prob, ties broken by lower index (matches jax.lax.top_k)
    top2 = np.argsort(-p, axis=-1, kind="stable")[:, :TOP_K]
    pt = np.take_along_axis(p, top2, axis=-1)
    wt = pt / pt.sum(axis=-1, keepdims=True)
    idxs, wts = [], []
    for e in range(NUM_EXPERTS):
        mask = top2 == e  # [T, 2]
        tok = np.nonzero(mask.any(axis=-1))[0]
        w = wt[tok, np.argmax(mask[tok], axis=-1)]
        idxs.append(tok)
        wts.append(w.astype(np.float32))
    return idxs, wts


def _chunks_for(load):
    """Split a token count into moving-dim chunks: all but the last are
    multiples of 128 in [256, 512]; keep the ragged tail >= 240 when
    possible (short moving dims go LDWEIGHTS-bound)."""
    C = load
    n = max(1, -(-C // 512))
    chunks = []
    rem = C
    for i in range(n - 1):
        c = min(512, -(-rem // ((n - i) * 128)) * 128)
        chunks.append(c)
        rem -= c
    while n > 1 and rem < 240 and chunks:
        for i in range(len(chunks)):
            if rem >= 240:
                break
            if chunks[i] > 256:
                chunks[i] -= 128
                rem += 128
        else:
            break
    chunks.append(rem)
    assert sum(chunks) == C and all(c > 0 for c in chunks)
    return tuple(chunks)


def _chunks_first(load):
    """Position-0 expert: 384-col first chunk — long enough that its
    phase A covers the boot cascade's w2 delivery (the startup window is
    DMA-byte-bound), short enough to keep the boot-A blob small."""
    if load >= 1136:
        return (384, 512) + _chunks_for(load - 896)
    if load >= 624:
        return (384,) + _chunks_for(load - 384)
    return _chunks_for(load)


def _plan(loads_pos):
    """Per-position chunk tuples + 128-aligned xT column offsets.
    loads_pos is already in processing order."""
    chunks_e, offs = [], []
    off = 0
    for pos, l in enumerate(loads_pos):
        chunks_e.append(_chunks_first(l) if pos == 0 else _chunks_for(l))
        offs.append(off)
        off += -(-l // 128) * 128
    return tuple(chunks_e), tuple(offs), off


# ------------------------------------------------------------- device IR ----
def _build(plan):
    """Per-core Bacc graph. plan = (chunks_e, offs, XWT)."""
    import concourse.bacc as bacc
    import concourse.bass as bass
    import concourse.mybir as mybir
    import concourse.tile as tile

    chunks_e, offs, XWT = plan
    XW_e = [-(-sum(ch) // 128) * 128 for ch in chunks_e]

    bf16 = mybir.dt.bfloat16
    f32 = mybir.dt.float32

    nc = bacc.Bacc("TRN2", target_bir_lowering=False, debug=False,
                   num_devices=N_CORES)

    xT_d = nc.dram_tensor("xT", [HIDDEN, XWT], bf16, kind="ExternalInput")
    # w1s/w3s host-pre-tiled per expert as [e, p, ft, dc, 128]; w2s as
    # [e, p, ft, HIDDEN] so every DMA line is fully contiguous
    w1_d = nc.dram_tensor("w1s", [NUM_EXPERTS, 128, FT, DC, 128], bf16,
                          kind="ExternalInput")
    w3_d = nc.dram_tensor("w3s", [NUM_EXPERTS, 128, FT, DC, 128], bf16,
                          kind="ExternalInput")
    w2_d = nc.dram_tensor("w2s", [NUM_EXPERTS, 128, FT, HIDDEN], bf16,
                          kind="ExternalInput")
    # y layout: per-chunk blocks, each [128, DT, chunk] per-partition
    # contiguous, so a 4-dt group DMA is 128 descriptors of 4*chunk*2B
    # (the flat [dt, p, chunk] layout needs 128 descs per dt => the last
    # chunk's 1024 output descriptors trail the final matmul by ~3us)
    y_d = nc.dram_tensor("y", [128, DT * XWT], bf16, kind="ExternalOutput")

    # boot blob sections (bf16 elems per partition); see module docstring.
    # x sections: chunk0 rides in A; section F is exactly chunk1's range;
    # G covers the remaining chunks (each chunk fully inside one section).
    c0 = chunks_e[0][0]
    L0 = sum(chunks_e[0])
    xm = c0 + chunks_e[0][1] if len(chunks_e[0]) >= 2 else L0
    SEC = [2 * DC * 128 + DC * c0]          # A: w1ft0 | w3ft0 | x chunk0
    SEC += [2 * DC * 128] * (FT - 1)        # B,C,D: ft1..3 pairs
    SEC += [FT * HIDDEN]                    # E: w2
    if xm > c0:
        SEC += [DC * (xm - c0)]             # F: x cols [c0, xm)
    if L0 > xm:
        SEC += [DC * (L0 - xm)]             # G: x cols [xm, L0)
    BW = sum(SEC)
    boot_d = nc.dram_tensor("boot", [128, BW], bf16, kind="ExternalInput")

    xT_v = xT_d.ap().rearrange("(dc p) c -> p dc c", p=128)

    with tile.TileContext(nc) as tc:
        with (
            tc.tile_pool(name="xe", bufs=2) as xep,
            tc.tile_pool(name="w13", bufs=2) as w13,
            tc.tile_pool(name="w2p", bufs=2) as w2p,
            tc.tile_pool(name="hp", bufs=2) as hp,
            tc.tile_pool(name="sil", bufs=4) as silp,
            tc.tile_pool(name="yo", bufs=8) as yop,
            tc.tile_pool(name="ps", bufs=2, space=bass.MemorySpace.PSUM) as ps,
            tc.tile_pool(name="yps", bufs=4, space=bass.MemorySpace.PSUM) as yps,
        ):
            xe_tiles = {}
            w13_tiles = {}
            w2_tiles = {}

            def load_w13(e):
                w1_sb = w13.tile([128, FT, DC, 128], bf16, tag="w1",
                                 name="w1_sb")
                w3_sb = w13.tile([128, FT, DC, 128], bf16, tag="w3",
                                 name="w3_sb")
                nc.sync.dma_start(w1_sb[:], w1_d.ap()[e])
                nc.sync.dma_start(w3_sb[:], w3_d.ap()[e])
                w13_tiles[e] = (w1_sb, w3_sb)

            def load_xe(e):
                xe_tiles[e] = xep.tile([128, DC, XW_e[e]], bf16,
                                       tag="xT", name="xe_sb")
                nc.sync.dma_start(
                    xe_tiles[e][:],
                    xT_v[:, :, offs[e]:offs[e] + XW_e[e]],
                )

            def load_w2(e):
                w2_sb = w2p.tile([128, FT, HIDDEN], bf16, tag="w2",
                                 name="w2_sb")
                nc.sync.dma_start(w2_sb[:], w2_d.ap()[e])
                w2_tiles[e] = w2_sb

            # ---- startup: cascaded boot sections, each one DMA of 128
            # descriptors, issued on one ring in strict consumption order
            # (the queues process in FIFO order, so any other section
            # would steal early bandwidth from the critical one).
            bts = []
            off_el = 0
            for si, w in enumerate(SEC):
                bt = w13.tile([128, w], bf16, tag=f"boot{si}", bufs=1,
                              name=f"boot{si}")
                nc.sync.dma_start(bt[:], boot_d.ap()[:, off_el:off_el + w])
                bts.append(bt)
                off_el += w

            def w1slice(e, ft, dc):
                if e == 0:
                    b = bts[ft]
                    return b[:, dc * 128:(dc + 1) * 128]
                return w13_tiles[e][0][:, ft, dc, :]

            def w3slice(e, ft, dc):
                if e == 0:
                    b = bts[ft]
                    base = DC * 128
                    return b[:, base + dc * 128:base + (dc + 1) * 128]
                return w13_tiles[e][1][:, ft, dc, :]

            def w2slice(e, ft, d0, d1):
                if e == 0:
                    return bts[FT][:, ft * HIDDEN + d0:ft * HIDDEN + d1]
                return w2_tiles[e][:, ft, d0:d1]

            def xslice(e, dc, t0, chunk):
                if e == 0:
                    if t0 < c0:
                        assert t0 + chunk <= c0
                        base = 2 * DC * 128 + dc * c0
                        return bts[0][:, base + t0:base + t0 + chunk]
                    if t0 < xm:
                        assert t0 + chunk <= xm
                        b = bts[FT + 1]
                        base = dc * (xm - c0) + (t0 - c0)
                        return b[:, base:base + chunk]
                    b = bts[FT + 2]
                    base = dc * (L0 - xm) + (t0 - xm)
                    return b[:, base:base + chunk]
                return xe_tiles[e][:, dc, t0:t0 + chunk]

            # HAM pre-warm: keep the PE busy from kernel entry (~7.6us)
            # until boot section A lands (~11.4us) so the clock is fully
            # ramped when real work starts.
            warm_sb = silp.tile([128, 128], bf16, tag="warm_in", bufs=1)
            nc.vector.memset(warm_sb[:], 0.0)
            warm_ps = ps.tile([128, 128], f32, tag="ph1", name="warm_ps")
            N_WARM = 52
            for i in range(N_WARM):
                nc.tensor.matmul(warm_ps[:], warm_sb[:], warm_sb[:],
                                 start=(i == 0), stop=(i == N_WARM - 1))

            cum_cols = 0
            for e in range(NUM_EXPERTS):
                t0 = 0
                nch = len(chunks_e[e])
                # which chunk carries the next-expert prefetches: for e0
                # use the second chunk (chunk0's DMA window is packed
                # with the boot cascade)
                pf_ci = (1 if nch > 1 else 0) if e == 0 else 0
                for ci, chunk in enumerate(chunks_e[e]):
                    hT = hp.tile([128, FT, chunk], bf16, tag="hT")
                    # ---- phase A ----
                    for ft in range(FT):
                        if ci == pf_ci and e + 1 < NUM_EXPERTS:
                            if ft == 2:
                                load_w13(e + 1)
                            elif ft == 3:
                                load_xe(e + 1)
                                load_w2(e + 1)
                        ph1 = ps.tile([128, chunk], f32, tag="ph1")
                        ph3 = ps.tile([128, chunk], f32, tag="ph3")
                        for dc in range(DC):
                            nc.tensor.matmul(
                                ph1[:],
                                w1slice(e, ft, dc),
                                xslice(e, dc, t0, chunk),
                                start=(dc == 0), stop=(dc == DC - 1),
                            )
                        for dc in range(DC):
                            nc.tensor.matmul(
                                ph3[:],
                                w3slice(e, ft, dc),
                                xslice(e, dc, t0, chunk),
                                start=(dc == 0), stop=(dc == DC - 1),
                            )
                        sil = silp.tile([128, chunk], bf16, tag="sil")
                        nc.scalar.activation(
                            sil[:], ph1[:], mybir.ActivationFunctionType.Silu
                        )
                        nc.vector.tensor_mul(hT[:, ft, :], sil[:], ph3[:])

                    # ---- phase B: yT[d, t] partial, unscaled ----
                    ybase = DT * cum_cols
                    ysb = None
                    for dt in range(DT):
                        if dt % 4 == 0:
                            ysb = yop.tile([128, 4, chunk], bf16, tag="ysb")
                        yp = yps.tile([128, chunk], f32, tag="yp")
                        for ft in range(FT):
                            nc.tensor.matmul(
                                yp[:],
                                w2slice(e, ft, dt * 128, (dt + 1) * 128),
                                hT[:, ft, :],
                                start=(ft == 0), stop=(ft == FT - 1),
                            )
                        # alternate copies between ScalarE and DVE so
                        # neither engine becomes the bottleneck (PSUM
                        # same-bank reads serialize, so no quartering)
                        if dt % 2 == 0:
                            nc.scalar.copy(ysb[:, dt % 4, :], yp[:])
                        else:
                            nc.vector.tensor_copy(ysb[:, dt % 4, :], yp[:])
                        if dt % 4 == 3:
                            g0 = ybase + (dt - 3) * chunk
                            nc.sync.dma_start(
                                y_d.ap()[:, g0:g0 + 4 * chunk]
                                .rearrange("p (j c) -> p j c", j=4),
                                ysb[:],
                            )
                    t0 += chunk
                    cum_cols += chunk
    nc.compile()
    return nc


def _get_nc(plan):
    if plan not in _nc_cache:
        _nc_cache[plan] = _build(plan)
    return _nc_cache[plan]


# ---------------------------------------------------------------- kernel ----
def kernel(hidden_states, gate_w, gate_b, w1, w3, w2, _trace=False):
    from concourse.bass_utils import run_bass_kernel_spmd

    B, S, D = hidden_states.shape
    T = B * S
    x = np.asarray(hidden_states, np.float32).reshape(T, D)
    idxs, wts = _route(x, np.asarray(gate_w, np.float32),
                       np.asarray(gate_b, np.float32))
    loads = [len(i) for i in idxs]

    # processing order: largest load first (boot chunking wants >=1008),
    # smallest tail chunk last (cheapest final drain)
    order = sorted(range(NUM_EXPERTS), key=lambda e: -loads[e])
    tail = {e: _chunks_for(loads[e])[-1] for e in order[1:]}
    last = min(order[1:], key=lambda e: tail[e])
    perm = [order[0]] + [e for e in order[1:] if e != last] + [last]

    idxs = [idxs[e] for e in perm]
    wts = [wts[e] for e in perm]
    loads = [loads[e] for e in perm]
    plan = _plan(loads)
    chunks_e, offs, XWT = plan
    nc = _get_nc(plan)

    # shared xT: every expert's tokens in its 128-aligned column range,
    # in processing order
    xT = np.zeros((D, XWT), _BF16)
    for e in range(NUM_EXPERTS):
        xT[:, offs[e]:offs[e] + loads[e]] = x[idxs[e]].T.astype(_BF16)

    # weights pre-tiled for all cores in one reshape/transpose, expert
    # axis permuted into processing order:
    # w1/w3 [E, D, F] -> [core, E, 128, ft, dc, 128]
    w1 = np.asarray(w1, np.float32)[perm]
    w3 = np.asarray(w3, np.float32)[perm]
    w2 = np.asarray(w2, np.float32)[perm]
    w1t = np.ascontiguousarray(
        w1.reshape(NUM_EXPERTS, DC, 128, N_CORES, FT, 128)
        .transpose(3, 0, 2, 4, 1, 5)).astype(_BF16)
    w3t = np.ascontiguousarray(
        w3.reshape(NUM_EXPERTS, DC, 128, N_CORES, FT, 128)
        .transpose(3, 0, 2, 4, 1, 5)).astype(_BF16)
    # w2 [E, F, D] -> [core, E, 128, ft, D]
    w2t = np.ascontiguousarray(
        w2.reshape(NUM_EXPERTS, N_CORES, FT, 128, HIDDEN)
        .transpose(1, 0, 3, 2, 4)).astype(_BF16)

    # boot blob (see _build): per-partition-contiguous pack of expert
    # pos-0's whole working set in consumption order
    c0 = chunks_e[0][0]
    L0 = sum(chunks_e[0])
    xm = c0 + chunks_e[0][1] if len(chunks_e[0]) >= 2 else L0

    def xsec(a, b):
        return np.ascontiguousarray(
            xT.reshape(DC, 128, XWT)[:, :, a:b]
            .transpose(1, 0, 2).reshape(128, DC * (b - a)))

    boots = []
    for c in range(N_CORES):
        secs = [w1t[c, 0, :, 0].reshape(128, DC * 128),
                w3t[c, 0, :, 0].reshape(128, DC * 128),
                xsec(0, c0)]
        for ft in range(1, FT):
            secs += [w1t[c, 0, :, ft].reshape(128, DC * 128),
                     w3t[c, 0, :, ft].reshape(128, DC * 128)]
        secs += [w2t[c, 0].reshape(128, FT * HIDDEN)]
        if xm > c0:
            secs += [xsec(c0, xm)]
        if L0 > xm:
            secs += [xsec(xm, L0)]
        boots.append(np.concatenate(secs, axis=1))

    in_maps = [{
        "xT": xT,
        "w1s": w1t[c],
        "w3s": w3t[c],
        "w2s": w2t[c],
        "boot": boots[c],
    } for c in range(N_CORES)]

    res = run_bass_kernel_spmd(nc, in_maps, core_ids=list(range(N_CORES)),
                               trace=_trace)

    y2 = res.results[0]["y"].astype(np.float32)
    for c in range(1, N_CORES):
        y2 += res.results[c]["y"].astype(np.float32)
    # y2 is per-chunk blocks [128, DT, chunk]; rebuild yT [HIDDEN, XWT]
    yT = np.zeros((D, XWT), np.float32)
    cum = 0
    for e in range(NUM_EXPERTS):
        t0 = 0
        for chunk in chunks_e[e]:
            blk = y2[:, DT * cum:DT * (cum + chunk)].reshape(128, DT, chunk)
            yT[:, offs[e] + t0:offs[e] + t0 + chunk] = (
                blk.transpose(1, 0, 2).reshape(D, chunk))
            t0 += chunk
            cum += chunk
    out = np.zeros((T, D), np.float32)
    for e in range(NUM_EXPERTS):
        tok, wt = idxs[e], wts[e]
        seg = yT[:, offs[e]:offs[e] + loads[e]].T  # [load, D]
        out[tok] += wt[:, None] * seg
    out = out.reshape(B, S, D)
    if _trace:
        return out, res
    return out


# revision 28
# speedup vs baseline: 1.0291x; 1.0052x over previous
"""Trainium2 Bass kernel for nn_BlockSparseMoE (top-2 of 8 experts, SwiGLU).

Strategy (8-way tensor-parallel over FFN):
  - Host: compute router (gate matmul + softmax + top-2 + renorm) in fp64,
    gather each expert's tokens into a contiguous column range of one
    shared xT matrix.
  - Device (SPMD x8): every core holds a 512-wide F-slice of ALL 8
    experts' w1/w3/w2 (same ~25 MB weight traffic as one full expert in
    the expert-parallel layout) and runs all 8192 token-expert pairs
    against its slice — exactly T*K/8 = 1024 pair-equivalents per core
    regardless of routing imbalance. Partial y outputs (transposed,
    unscaled) stream back.
  - Host: sum the 8 partial outputs, scale by the renormalized top-2
    weight, scatter-add per token.

Per-core layout:
  phase A: hT[f, t] = silu(x@w1)^T * (x@w3)^T per expert (FT=4 f-tiles of
           128), lhsT = w1 tile [128, 128f], rhs = xT d-chunk
           [128, tchunk] — weights stationary, tokens moving.
  phase B: yT[d, t] = w2_slice^T @ hT, lhsT = w2 f-tile [128f, 128d],
           rhs = hT f-tile [128, tchunk] — tokens moving, so ragged
           expert tails cost no extra PE cycles; no on-device scaling.

Startup: the DMA queues only start fetching descriptors ~8.6us into the
  kernel and ramp slowly, and every DMA into a 128-partition SBUF tile
  costs >=128 descriptors. So the first expert's entire working set is
  packed into ONE per-partition-contiguous "boot" DRAM blob, split into
  cascaded 128-descriptor sections sized so each lands just before the
  compute that needs it: [w1ft0|w3ft0|x-chunk0(256)] -> ft1 pair -> ft2
  pair -> ft3 pair -> w2 -> x cols 256:768 -> x rest. A short HAM warmup
  bridges kernel entry to the first section landing (~11.4us).

Expert order: processed by descending load (position 0 needs >=1008
  tokens for the 256/512 boot chunking to apply), except the expert with
  the smallest tail chunk goes last so the final drain is cheapest.
"""

import numpy as np
import ml_dtypes

HIDDEN = 1024
FFN = 4096
NUM_EXPERTS = 8
TOP_K = 2
N_CORES = 8
FS = FFN // N_CORES          # 512-wide F-slice per core
DC = HIDDEN // 128           # 8 contraction chunks for x@w1
FT = FS // 128               # 4 f-tiles per expert slice
DT = HIDDEN // 128           # 8 output d-tiles

_BF16 = ml_dtypes.bfloat16
_nc_cache = {}


# ---------------------------------------------------------------- router ----
def _route(x, gate_w, gate_b):
    """Top-2 routing. Returns per-expert (token_idx, renorm_weight)."""
    logits = x.astype(np.float64) @ gate_w.astype(np.float64) + gate_b.astype(
        np.float64
    )
    logits -= logits.max(axis=-1, keepdims=True)
    p = np.exp(logits)
    p /= p.sum(axis=-1, keepdims=True)
    # top-2 by prob, ties broken by lower index (matches jax.lax.top_k)
    top2 = np.argsort(-p, axis=-1, kind="stable")[:, :TOP_K]
    pt = np.take_along_axis(p, top2, axis=-1)
    wt = pt / pt.sum(axis=-1, keepdims=True)
    idxs, wts = [], []
    for e in range(NUM_EXPERTS):
        mask = top2 == e  # [T, 2]
        tok = np.nonzero(mask.any(axis=-1))[0]
        w = wt[tok, np.argmax(mask[tok], axis=-1)]
        idxs.append(tok)
        wts.append(w.astype(np.float32))
    return idxs, wts


def _chunks_for(load):
    """Split a token count into moving-dim chunks: all but the last are
    multiples of 128 in [256, 512]; keep the ragged tail >= 240 when
    possible (short moving dims go LDWEIGHTS-bound)."""
    C = load
    n = max(1, -(-C // 512))
    chunks = []
    rem = C
    for i in range(n - 1):
        c = min(512, -(-rem // ((n - i) * 128)) * 128)
        chunks.append(c)
        rem -= c
    while n > 1 and rem < 240 and chunks:
        for i in range(len(chunks)):
            if rem >= 240:
                break
            if chunks[i] > 256:
                chunks[i] -= 128
                rem += 128
        else:
            break
    chunks.append(rem)
    assert sum(chunks) == C and all(c > 0 for c in chunks)
    return tuple(chunks)


def _chunks_first(load):
    """Position-0 expert: 384-col first chunk — long enough that its
    phase A covers the boot cascade's w2 delivery (the startup window is
    DMA-byte-bound), short enough to keep the boot-A blob small."""
    if load >= 1136:
        return (384, 512) + _chunks_for(load - 896)
    if load >= 624:
        return (384,) + _chunks_for(load - 384)
    return _chunks_for(load)


def _plan(loads_pos):
    """Per-position chunk tuples + 128-aligned xT column offsets.
    loads_pos is already in processing order."""
    chunks_e, offs = [], []
    off = 0
    for pos, l in enumerate(loads_pos):
        chunks_e.append(_chunks_first(l) if pos == 0 else _chunks_for(l))
        offs.append(off)
        off += -(-l // 128) * 128
    return tuple(chunks_e), tuple(offs), off


# ------------------------------------------------------------- device IR ----
def _build(plan):
    """Per-core Bacc graph. plan = (chunks_e, offs, XWT)."""
    import concourse.bacc as bacc
    import concourse.bass as bass
    import concourse.mybir as mybir
    import concourse.tile as tile

    chunks_e, offs, XWT = plan
    XW_e = [-(-sum(ch) // 128) * 128 for ch in chunks_e]

    bf16 = mybir.dt.bfloat16
    f32 = mybir.dt.float32

    nc = bacc.Bacc("TRN2", target_bir_lowering=False, debug=False,
                   num_devices=N_CORES)

    xT_d = nc.dram_tensor("xT", [HIDDEN, XWT], bf16, kind="ExternalInput")
    # w1s/w3s host-pre-tiled per expert as [e, p, ft, dc, 128]; w2s as
    # [e, p, ft, HIDDEN] so every DMA line is fully contiguous
    w1_d = nc.dram_tensor("w1s", [NUM_EXPERTS, 128, FT, DC, 128], bf16,
                          kind="ExternalInput")
    w3_d = nc.dram_tensor("w3s", [NUM_EXPERTS, 128, FT, DC, 128], bf16,
                          kind="ExternalInput")
    w2_d = nc.dram_tensor("w2s", [NUM_EXPERTS, 128, FT, HIDDEN], bf16,
                          kind="ExternalInput")
    # y layout: per-chunk blocks, each [128, DT, chunk] per-partition
    # contiguous, so a 4-dt group DMA is 128 descriptors of 4*chunk*2B
    # (the flat [dt, p, chunk] layout needs 128 descs per dt => the last
    # chunk's 1024 output descriptors trail the final matmul by ~3us)
    y_d = nc.dram_tensor("y", [128, DT * XWT], bf16, kind="ExternalOutput")

    # boot blob sections (bf16 elems per partition); see module docstring.
    # x sections: chunk0 rides in A; section F is exactly chunk1's range;
    # G covers the remaining chunks (each chunk fully inside one section).
    c0 = chunks_e[0][0]
    L0 = sum(chunks_e[0])
    xm = c0 + chunks_e[0][1] if len(chunks_e[0]) >= 2 else L0
    SEC = [DC * 128 + DC * c0]              # A: w1ft0 | x chunk0
    SEC += [DC * 128]                       # B: w3ft0
    SEC += [2 * DC * 128] * (FT - 1)        # C,D,E: ft1..3 pairs
    SEC += [FT * HIDDEN]                    # F: w2
    if xm > c0:
        SEC += [DC * (xm - c0)]             # G: x cols [c0, xm)
    if L0 > xm:
        SEC += [DC * (L0 - xm)]             # H: x cols [xm, L0)
    BW = sum(SEC)
    boot_d = nc.dram_tensor("boot", [128, BW], bf16, kind="ExternalInput")

    xT_v = xT_d.ap().rearrange("(dc p) c -> p dc c", p=128)

    with tile.TileContext(nc) as tc:
        with (
            tc.tile_pool(name="xe", bufs=2) as xep,
            tc.tile_pool(name="w13", bufs=2) as w13,
            tc.tile_pool(name="w2p", bufs=2) as w2p,
            tc.tile_pool(name="hp", bufs=2) as hp,
            tc.tile_pool(name="sil", bufs=4) as silp,
            tc.tile_pool(name="yo", bufs=8) as yop,
            tc.tile_pool(name="ps", bufs=2, space=bass.MemorySpace.PSUM) as ps,
            tc.tile_pool(name="yps", bufs=4, space=bass.MemorySpace.PSUM) as yps,
        ):
            xe_tiles = {}
            w13_tiles = {}
            w2_tiles = {}

            def load_w13(e):
                w1_sb = w13.tile([128, FT, DC, 128], bf16, tag="w1",
                                 name="w1_sb")
                w3_sb = w13.tile([128, FT, DC, 128], bf16, tag="w3",
                                 name="w3_sb")
                nc.sync.dma_start(w1_sb[:], w1_d.ap()[e])
                nc.sync.dma_start(w3_sb[:], w3_d.ap()[e])
                w13_tiles[e] = (w1_sb, w3_sb)

            def load_xe(e):
                xe_tiles[e] = xep.tile([128, DC, XW_e[e]], bf16,
                                       tag="xT", name="xe_sb")
                nc.sync.dma_start(
                    xe_tiles[e][:],
                    xT_v[:, :, offs[e]:offs[e] + XW_e[e]],
                )

            def load_w2(e):
                w2_sb = w2p.tile([128, FT, HIDDEN], bf16, tag="w2",
                                 name="w2_sb")
                nc.sync.dma_start(w2_sb[:], w2_d.ap()[e])
                w2_tiles[e] = w2_sb

            # ---- startup: cascaded boot sections, each one DMA of 128
            # descriptors, issued on one ring in strict consumption order
            # (the queues process in FIFO order, so any other section
            # would steal early bandwidth from the critical one).
            bts = []
            off_el = 0
            for si, w in enumerate(SEC):
                bt = w13.tile([128, w], bf16, tag=f"boot{si}", bufs=1,
                              name=f"boot{si}")
                nc.sync.dma_start(bt[:], boot_d.ap()[:, off_el:off_el + w])
                bts.append(bt)
                off_el += w

            def w1slice(e, ft, dc):
                if e == 0:
                    b = bts[0] if ft == 0 else bts[ft + 1]
                    return b[:, dc * 128:(dc + 1) * 128]
                return w13_tiles[e][0][:, ft, dc, :]

            def w3slice(e, ft, dc):
                if e == 0:
                    if ft == 0:
                        return bts[1][:, dc * 128:(dc + 1) * 128]
                    base = DC * 128
                    return bts[ft + 1][:, base + dc * 128:
                                       base + (dc + 1) * 128]
                return w13_tiles[e][1][:, ft, dc, :]

            def w2slice(e, ft, d0, d1):
                if e == 0:
                    return bts[FT + 1][:, ft * HIDDEN + d0:ft * HIDDEN + d1]
                return w2_tiles[e][:, ft, d0:d1]

            def xslice(e, dc, t0, chunk):
                if e == 0:
                    if t0 < c0:
                        assert t0 + chunk <= c0
                        base = DC * 128 + dc * c0
                        return bts[0][:, base + t0:base + t0 + chunk]
                    if t0 < xm:
                        assert t0 + chunk <= xm
                        b = bts[FT + 2]
                        base = dc * (xm - c0) + (t0 - c0)
                        return b[:, base:base + chunk]
                    b = bts[FT + 3]
                    base = dc * (L0 - xm) + (t0 - xm)
                    return b[:, base:base + chunk]
                return xe_tiles[e][:, dc, t0:t0 + chunk]

            # HAM pre-warm: keep the PE busy from kernel entry (~7.6us)
            # until boot section A lands (~11.4us) so the clock is fully
            # ramped when real work starts.
            warm_sb = silp.tile([128, 128], bf16, tag="warm_in", bufs=1)
            nc.vector.memset(warm_sb[:], 0.0)
            warm_ps = ps.tile([128, 128], f32, tag="ph1", name="warm_ps")
            N_WARM = 48
            for i in range(N_WARM):
                nc.tensor.matmul(warm_ps[:], warm_sb[:], warm_sb[:],
                                 start=(i == 0), stop=(i == N_WARM - 1))

            cum_cols = 0
            for e in range(NUM_EXPERTS):
                t0 = 0
                nch = len(chunks_e[e])
                # which chunk carries the next-expert prefetches: for e0
                # use the second chunk (chunk0's DMA window is packed
                # with the boot cascade)
                pf_ci = (1 if nch > 1 else 0) if e == 0 else 0
                for ci, chunk in enumerate(chunks_e[e]):
                    hT = hp.tile([128, FT, chunk], bf16, tag="hT")
                    # ---- phase A ----
                    for ft in range(FT):
                        if ci == pf_ci and e + 1 < NUM_EXPERTS:
                            if ft == 2:
                                load_w13(e + 1)
                            elif ft == 3:
                                load_xe(e + 1)
                                load_w2(e + 1)
                        ph1 = ps.tile([128, chunk], f32, tag="ph1")
                        ph3 = ps.tile([128, chunk], f32, tag="ph3")
                        for dc in range(DC):
                            nc.tensor.matmul(
                                ph1[:],
                                w1slice(e, ft, dc),
                                xslice(e, dc, t0, chunk),
                                start=(dc == 0), stop=(dc == DC - 1),
                            )
                        for dc in range(DC):
                            nc.tensor.matmul(
                                ph3[:],
                                w3slice(e, ft, dc),
                                xslice(e, dc, t0, chunk),
                                start=(dc == 0), stop=(dc == DC - 1),
                            )
                        sil = silp.tile([128, chunk], bf16, tag="sil")
                        nc.scalar.activation(
                            sil[:], ph1[:], mybir.ActivationFunctionType.Silu
                        )
                        nc.vector.tensor_mul(hT[:, ft, :], sil[:], ph3[:])

                    # ---- phase B: yT[d, t] partial, unscaled ----
                    ybase = DT * cum_cols
                    is_last = (e == NUM_EXPERTS - 1 and ci == nch - 1)
                    # y DMAs grouped 4 dt-tiles at a time (128 descriptors
                    # of 4*chunk*2B instead of 4x128 short ones); the very
                    # last chunk uses pairs so the final DMA trailing the
                    # last matmul is half as large
                    gsz = 2 if is_last else 4
                    ysb = None
                    for dt in range(DT):
                        if dt % gsz == 0:
                            ysb = yop.tile([128, gsz, chunk], bf16,
                                           tag="ysb2" if is_last else "ysb")
                        yp = yps.tile([128, chunk], f32, tag="yp")
                        for ft in range(FT):
                            nc.tensor.matmul(
                                yp[:],
                                w2slice(e, ft, dt * 128, (dt + 1) * 128),
                                hT[:, ft, :],
                                start=(ft == 0), stop=(ft == FT - 1),
                            )
                        # alternate copies between ScalarE and DVE so
                        # neither engine becomes the bottleneck (PSUM
                        # same-bank reads serialize, so no quartering)
                        if dt % 2 == 0:
                            nc.scalar.copy(ysb[:, dt % gsz, :], yp[:])
                        else:
                            nc.vector.tensor_copy(ysb[:, dt % gsz, :], yp[:])
                        if dt % gsz == gsz - 1:
                            g0 = ybase + (dt - gsz + 1) * chunk
                            nc.sync.dma_start(
                                y_d.ap()[:, g0:g0 + gsz * chunk]
                                .rearrange("p (j c) -> p j c", j=gsz),
                                ysb[:],
                            )
                    t0 += chunk
                    cum_cols += chunk
    nc.compile()
    return nc


def _get_nc(plan):
    if plan not in _nc_cache:
        _nc_cache[plan] = _build(plan)
    return _nc_cache[plan]


# ---------------------------------------------------------------- kernel ----
def kernel(hidden_states, gate_w, gate_b, w1, w3, w2, _trace=False):
    from concourse.bass_utils import run_bass_kernel_spmd

    B, S, D = hidden_states.shape
    T = B * S
    x = np.asarray(hidden_states, np.float32).reshape(T, D)
    idxs, wts = _route(x, np.asarray(gate_w, np.float32),
                       np.asarray(gate_b, np.float32))
    loads = [len(i) for i in idxs]

    # processing order: largest load first (boot chunking wants >=1008),
    # smallest tail chunk last (cheapest final drain)
    order = sorted(range(NUM_EXPERTS), key=lambda e: -loads[e])
    tail = {e: _chunks_for(loads[e])[-1] for e in order[1:]}
    last = min(order[1:], key=lambda e: tail[e])
    perm = [order[0]] + [e for e in order[1:] if e != last] + [last]

    idxs = [idxs[e] for e in perm]
    wts = [wts[e] for e in perm]
    loads = [loads[e] for e in perm]
    plan = _plan(loads)
    chunks_e, offs, XWT = plan
    nc = _get_nc(plan)

    # shared xT: every expert's tokens in its 128-aligned column range,
    # in processing order
    xT = np.zeros((D, XWT), _BF16)
    for e in range(NUM_EXPERTS):
        xT[:, offs[e]:offs[e] + loads[e]] = x[idxs[e]].T.astype(_BF16)

    # weights pre-tiled for all cores in one reshape/transpose, expert
    # axis permuted into processing order:
    # w1/w3 [E, D, F] -> [core, E, 128, ft, dc, 128]
    w1 = np.asarray(w1, np.float32)[perm]
    w3 = np.asarray(w3, np.float32)[perm]
    w2 = np.asarray(w2, np.float32)[perm]
    w1t = np.ascontiguousarray(
        w1.reshape(NUM_EXPERTS, DC, 128, N_CORES, FT, 128)
        .transpose(3, 0, 2, 4, 1, 5)).astype(_BF16)
    w3t = np.ascontiguousarray(
        w3.reshape(NUM_EXPERTS, DC, 128, N_CORES, FT, 128)
        .transpose(3, 0, 2, 4, 1, 5)).astype(_BF16)
    # w2 [E, F, D] -> [core, E, 128, ft, D]
    w2t = np.ascontiguousarray(
        w2.reshape(NUM_EXPERTS, N_CORES, FT, 128, HIDDEN)
        .transpose(1, 0, 3, 2, 4)).astype(_BF16)

    # boot blob (see _build): per-partition-contiguous pack of expert
    # pos-0's whole working set in consumption order
    c0 = chunks_e[0][0]
    L0 = sum(chunks_e[0])
    xm = c0 + chunks_e[0][1] if len(chunks_e[0]) >= 2 else L0

    def xsec(a, b):
        return np.ascontiguousarray(
            xT.reshape(DC, 128, XWT)[:, :, a:b]
            .transpose(1, 0, 2).reshape(128, DC * (b - a)))

    boots = []
    for c in range(N_CORES):
        secs = [w1t[c, 0, :, 0].reshape(128, DC * 128),
                xsec(0, c0),
                w3t[c, 0, :, 0].reshape(128, DC * 128)]
        for ft in range(1, FT):
            secs += [np.concatenate(
                [w1t[c, 0, :, ft].reshape(128, DC * 128),
                 w3t[c, 0, :, ft].reshape(128, DC * 128)], axis=1)]
        secs += [w2t[c, 0].reshape(128, FT * HIDDEN)]
        if xm > c0:
            secs += [xsec(c0, xm)]
        if L0 > xm:
            secs += [xsec(xm, L0)]
        boots.append(np.concatenate(secs, axis=1))

    in_maps = [{
        "xT": xT,
        "w1s": w1t[c],
        "w3s": w3t[c],
        "w2s": w2t[c],
        "boot": boots[c],
    } for c in range(N_CORES)]

    res = run_bass_kernel_spmd(nc, in_maps, core_ids=list(range(N_CORES)),
                               trace=_trace)

    y2 = res.results[0]["y"].astype(np.float32)
    for c in range(1, N_CORES):
        y2 += res.results[c]["y"].astype(np.float32)
    # y2 is per-chunk blocks [128, DT, chunk]; rebuild yT [HIDDEN, XWT]
    yT = np.zeros((D, XWT), np.float32)
    cum = 0
    for e in range(NUM_EXPERTS):
        t0 = 0
        for chunk in chunks_e[e]:
            blk = y2[:, DT * cum:DT * (cum + chunk)].reshape(128, DT, chunk)
            yT[:, offs[e] + t0:offs[e] + t0 + chunk] = (
                blk.transpose(1, 0, 2).reshape(D, chunk))
            t0 += chunk
            cum += chunk
    out = np.zeros((T, D), np.float32)
    for e in range(NUM_EXPERTS):
        tok, wt = idxs[e], wts[e]
        seg = yT[:, offs[e]:offs[e] + loads[e]].T  # [load, D]
        out[tok] += wt[:, None] * seg
    out = out.reshape(B, S, D)
    if _trace:
        return out, res
    return out
